# revision 1
# baseline (speedup 1.0000x reference)
"""AttnBlock (GroupNorm -> QKV -> 4096x4096 single-head attention -> proj ->
residual) on 8 TRN2 NeuronCores.

Sharding: data-parallel over batch (B=2) x sequence-parallel over query
positions (4 slabs of 1024). Each core receives the full x[b] (rolled so its
query slab sits at columns 0:1024), computes GroupNorm stats + k/v for the
whole image (replicated within the 4-core batch group -> zero collectives),
and attention + projection + residual for its 1024 query columns only.

Precision: matmuls in bf16 with fp32 PSUM accumulation; GroupNorm stats,
softmax sums and the residual path in fp32.
"""
import sys
sys.path.insert(0, '/opt/trn_rl_repo')
import contextlib
import numpy as np
import ml_dtypes

import concourse.bass as bass
import concourse.tile as tile
from concourse import mybir, bacc
from concourse import bass_utils

f32 = mybir.dt.float32
bf16 = mybir.dt.bfloat16
AF = mybir.ActivationFunctionType
ALU = mybir.AluOpType

C = 512          # channels
N = 4096         # positions (64*64)
G = 32           # groupnorm groups
GP = 16          # channels per group
NT = C // 128    # 4 channel partition-tiles
QS = 1024        # query slab per core
EPS = 1e-6
NELEM = float(GP * N)   # elements per group


def _make_pools(tc, ctx):
    p = {}
    p["singles"] = ctx.enter_context(tc.tile_pool(name="singles", bufs=1))
    p["sq_scr"] = ctx.enter_context(tc.tile_pool(name="sq_scr", bufs=2))
    p["xbf"] = ctx.enter_context(tc.tile_pool(name="xbf", bufs=1))
    p["xslab"] = ctx.enter_context(tc.tile_pool(name="xslab", bufs=1))
    p["ksb"] = ctx.enter_context(tc.tile_pool(name="ksb", bufs=1))
    p["vtsb"] = ctx.enter_context(tc.tile_pool(name="vtsb", bufs=1))
    p["qsb"] = ctx.enter_context(tc.tile_pool(name="qsb", bufs=1))
    p["psb"] = ctx.enter_context(tc.tile_pool(name="psb", bufs=3))
    p["onsb"] = ctx.enter_context(tc.tile_pool(name="onsb", bufs=8))
    p["outsb"] = ctx.enter_context(tc.tile_pool(name="outsb", bufs=2))
    p["small"] = ctx.enter_context(tc.tile_pool(name="small", bufs=1))
    p["ps_acc"] = ctx.enter_context(tc.tile_pool(name="ps_acc", bufs=1, space="PSUM"))
    p["ps_st"] = ctx.enter_context(tc.tile_pool(name="ps_st", bufs=3, space="PSUM"))
    return p


def _emit_body(nc, tc, p, xbf_d, xslab_d, wall_d, misc_d, out):
    singles, sq_scr, xbf, xslab = p["singles"], p["sq_scr"], p["xbf"], p["xslab"]
    ksb, vtsb, qsb, psb = p["ksb"], p["vtsb"], p["qsb"], p["psb"]
    onsb, outsb, small = p["onsb"], p["outsb"], p["small"]
    ps_acc, ps_st = p["ps_acc"], p["ps_st"]

    # ---- phase 0: x chunks first (critical path), then packed inputs
    x_bf = [xbf.tile([128, N], bf16, name=f"xbf{t}", tag=f"xbf{t}")
            for t in range(NT)]
    sum_parts = small.tile([128, 8], f32, tag="sum_parts")
    sq_parts = small.tile([128, 8], f32, tag="sq_parts")
    xchunk = []
    for i in range(8):
        t, j = i // 2, i % 2
        sl = x_bf[t][:, j * 2048:(j + 1) * 2048]
        nc.sync.dma_start(sl, xbf_d.ap()[t * 128:(t + 1) * 128,
                                         j * 2048:(j + 1) * 2048])
        xchunk.append(sl)
    wall_t = singles.tile([128, 16 * 512], bf16, tag="wall")
    nc.sync.dma_start(wall_t[:], wall_d.ap())
    misc_t = singles.tile([128, 668], f32, tag="misc")
    nc.sync.dma_start(misc_t[:], misc_d.ap())
    x_sl = []
    for t in range(NT):
        xs = xslab.tile([128, QS], f32, tag=f"xsl{t}")
        nc.sync.dma_start(xs[:], xslab_d.ap()[t * 128:(t + 1) * 128, :])
        x_sl.append(xs)

    # views into the packed tiles
    w_t = {}
    for iw, name in enumerate(("q", "k", "v", "p")):
        w_t[name] = [wall_t[:, (iw * 4 + t) * 512:(iw * 4 + t + 1) * 512]
                     for t in range(NT)]
    bqk_t = [misc_t[:, 2 * t:2 * t + 2] for t in range(NT)]
    bp_t = [misc_t[:, 8 + t:9 + t] for t in range(NT)]
    gnw_t = [misc_t[:, 12 + t:13 + t] for t in range(NT)]
    gnb_t = [misc_t[:, 16 + t:17 + t] for t in range(NT)]
    sel8_t = misc_t[:, 20:28]
    sel8T_t = misc_t[0:8, 28:156]
    bvr = misc_t[0:1, 156:668]

    ones_col = singles.tile([128, 1], bf16, tag="ones_col")
    nc.vector.memset(ones_col[:], 1.0)
    ones_row = singles.tile([1, 128], bf16, tag="ones_row")
    nc.vector.memset(ones_row[:], 1.0)
    eps8 = singles.tile([8, 1], f32, tag="eps8")
    nc.vector.memset(eps8[:], EPS)

    # stats per chunk (bf16 input, fp32 accumulation)
    for i in range(8):
        sl = xchunk[i]
        nc.vector.reduce_sum(sum_parts[:, i:i + 1], sl,
                             axis=mybir.AxisListType.X)
        sq = sq_scr.tile([128, 2048], bf16, tag="sq")
        nc.scalar.activation(sq[:], sl, AF.Square,
                             accum_out=sq_parts[:, i:i + 1])

    # ---- phase 0c: finalize group stats --------------------------
    stats_both = small.tile([128, 8], f32, tag="stats_both")
    nc.vector.reduce_sum(stats_both[:, 0:4],
                         sum_parts[:].rearrange("p (t j) -> p t j", j=2),
                         axis=mybir.AxisListType.X)
    nc.vector.reduce_sum(stats_both[:, 4:8],
                         sq_parts[:].rearrange("p (t j) -> p t j", j=2),
                         axis=mybir.AxisListType.X)
    # group-reduce over the 16-channel groups: psum_g[8, 2t:2t+2]
    ps_g = ps_st.tile([8, 8], f32, tag="st")
    for t in range(NT):
        nc.tensor.matmul(ps_g[:, 2 * t:2 * t + 2], sel8_t,
                         stats_both[:, t::4], start=True, stop=True)
    mstats = small.tile([8, 8], f32, tag="mstats")
    nc.scalar.mul(mstats[:], ps_g[:], 1.0 / NELEM)
    mean_v = mstats[:, 0::2]
    ex2_v = mstats[:, 1::2]
    var8 = small.tile([8, 4], f32, tag="var8")
    m2 = small.tile([8, 4], f32, tag="m2")
    nc.vector.tensor_tensor(m2[:], mean_v, mean_v, op=ALU.mult)
    nc.vector.tensor_tensor(var8[:], ex2_v, m2[:], op=ALU.subtract)
    lnv = small.tile([8, 4], f32, tag="lnv")
    nc.scalar.activation(lnv[:], var8[:], AF.Ln, bias=eps8[:])
    rstd8 = small.tile([8, 4], f32, tag="rstd8")
    nc.scalar.activation(rstd8[:], lnv[:], AF.Exp, scale=-0.5)
    grp2 = small.tile([8, 8], f32, tag="grp2")
    nc.vector.tensor_copy(grp2[:, 0::2], mean_v)
    nc.vector.tensor_copy(grp2[:, 1::2], rstd8[:])
    # broadcast group -> channel: bc[128, 2] = (mean_c, rstd_c)
    scale_t, shift_bf = [], []
    for t in range(NT):
        ps_bc = ps_st.tile([128, 2], f32, tag="st")
        nc.tensor.matmul(ps_bc[:], sel8T_t, grp2[0:8, 2 * t:2 * t + 2],
                         start=True, stop=True)
        sc = small.tile([128, 1], f32, tag=f"scale{t}")
        nc.vector.tensor_tensor(sc[:], gnw_t[t], ps_bc[:, 1:2], op=ALU.mult)
        scale_t.append(sc)
        nsc = small.tile([128, 1], f32, tag=f"nscale{t}")
        nc.vector.tensor_scalar_mul(nsc[:], sc[:], -1.0)
        sh = small.tile([128, 1], f32, tag=f"shift{t}")
        nc.vector.scalar_tensor_tensor(sh[:], ps_bc[:, 0:1], nsc[:],
                                       gnb_t[t], op0=ALU.mult, op1=ALU.add)
        shb = small.tile([128, 1], bf16, tag=f"shiftb{t}")
        nc.vector.tensor_copy(shb[:], sh[:])
        shift_bf.append(shb)

    # ---- phase 0d: fold GN into weights and biases ---------------
    b2qk_t = []
    for t in range(NT):
        ps_b = ps_st.tile([128, 2], f32, tag="st")
        for j, wname in enumerate(("q", "k")):
            for ct in range(NT):
                nc.tensor.matmul(ps_b[:, j:j + 1],
                                 w_t[wname][ct][:, t * 128:(t + 1) * 128],
                                 shift_bf[ct][:],
                                 start=(ct == 0), stop=(ct == 3))
        b2 = small.tile([128, 2], f32, tag=f"b2qk{t}")
        nc.vector.tensor_tensor(b2[:], ps_b[:], bqk_t[t], op=ALU.add)
        b2qk_t.append(b2)
    ps_vr = ps_st.tile([1, C], f32, tag="st")
    for ct in range(NT):
        nc.tensor.matmul(ps_vr[:], shift_bf[ct][:], w_t["v"][ct],
                         start=(ct == 0), stop=(ct == 3))
    bv2 = small.tile([1, C], f32, tag="bv2")
    nc.vector.tensor_tensor(bv2[:], ps_vr[:], bvr, op=ALU.add)
    bv2b = small.tile([1, C], bf16, tag="bv2b")
    nc.vector.tensor_copy(bv2b[:], bv2[:])
    # broadcast v-bias across partitions once (vs a K=1 matmul per n-tile)
    ps_bb = ps_st.tile([128, C], f32, tag="st")
    nc.tensor.matmul(ps_bb[:], ones_row[:], bv2b[:], start=True, stop=True)
    bv_bc = singles.tile([128, C], bf16, tag="bv_bc")
    nc.scalar.copy(bv_bc[:], ps_bb[:])
    # scale folds (in place on the weight tiles)
    for wname in ("q", "k", "v"):
        for ct in range(NT):
            nc.vector.tensor_scalar_mul(w_t[wname][ct], w_t[wname][ct],
                                        scale_t[ct][:])

    # ---- phase 1: q, k, vT projections ---------------------------
    q_sb = [qsb.tile([128, QS], bf16, name=f"q{t}", tag=f"q{t}")
            for t in range(NT)]
    for t in range(NT):
        for nch in range(QS // 512):
            ps = ps_st.tile([128, 512], f32, tag="st")
            for ct in range(NT):
                nc.tensor.matmul(ps[:],
                                 w_t["q"][ct][:, t * 128:(t + 1) * 128],
                                 x_bf[ct][:, nch * 512:(nch + 1) * 512],
                                 start=(ct == 0), stop=(ct == 3))
            nc.scalar.activation(q_sb[t][:, nch * 512:(nch + 1) * 512],
                                 ps[:], AF.Identity, bias=b2qk_t[t][:, 0:1])
    k_sb = [ksb.tile([128, N], bf16, name=f"k{t}", tag=f"k{t}")
            for t in range(NT)]
    for t in range(NT):
        for nch in range(N // 512):
            ps = ps_st.tile([128, 512], f32, tag="st")
            for ct in range(NT):
                nc.tensor.matmul(ps[:],
                                 w_t["k"][ct][:, t * 128:(t + 1) * 128],
                                 x_bf[ct][:, nch * 512:(nch + 1) * 512],
                                 start=(ct == 0), stop=(ct == 3))
            nc.scalar.activation(k_sb[t][:, nch * 512:(nch + 1) * 512],
                                 ps[:], AF.Identity, bias=b2qk_t[t][:, 1:2])
    vt_sb = [vtsb.tile([128, C], bf16, name=f"vt{nt}", tag=f"vt{nt}")
             for nt in range(N // 128)]
    for nt in range(N // 128):
        ps = ps_st.tile([128, C], f32, tag="st")
        for ct in range(NT):
            nc.tensor.matmul(ps[:],
                             x_bf[ct][:, nt * 128:(nt + 1) * 128],
                             w_t["v"][ct],
                             start=(ct == 0), stop=(ct == NT - 1))
        nc.vector.tensor_tensor(vt_sb[nt][:], ps[:], bv_bc[:], op=ALU.add)

    # ---- phase 2: attention + projection per 512-query chunk -----
    for qch in range(QS // 512):
        o_ps = [ps_acc.tile([128, 512], f32, name=f"o{t}", tag=f"o{t}")
                for t in range(NT)]
        sums_ps = ps_acc.tile([1, 512], f32, tag="sums")
        for kt in range(N // 128):
            st_ps = ps_st.tile([128, 512], f32, tag="st")
            for ct in range(NT):
                nc.tensor.matmul(st_ps[:],
                                 k_sb[ct][:, kt * 128:(kt + 1) * 128],
                                 q_sb[ct][:, qch * 512:(qch + 1) * 512],
                                 start=(ct == 0), stop=(ct == 3))
            p_t = psb.tile([128, 512], bf16, tag="p")
            nc.scalar.activation(p_t[:], st_ps[:], AF.Exp)
            for ct in range(NT):
                nc.tensor.matmul(o_ps[ct][:],
                                 vt_sb[kt][:, ct * 128:(ct + 1) * 128],
                                 p_t[:],
                                 start=(kt == 0), stop=(kt == N // 128 - 1))
            nc.tensor.matmul(sums_ps[:], ones_col[:], p_t[:],
                             start=(kt == 0), stop=(kt == N // 128 - 1))
        o_n = []
        for ct in range(NT):
            on = onsb.tile([128, 512], bf16, tag="on")
            nc.vector.tensor_copy(on[:], o_ps[ct][:])
            o_n.append(on)
        r_row = small.tile([1, 512], f32, tag="r_row")
        nc.vector.reciprocal(r_row[:], sums_ps[:])
        r_bf = small.tile([1, 512], bf16, tag="r_bf")
        nc.vector.tensor_copy(r_bf[:], r_row[:])
        ps_rb = ps_st.tile([128, 512], f32, tag="st")
        nc.tensor.matmul(ps_rb[:], ones_row[:], r_bf[:], start=True, stop=True)
        r_all = small.tile([128, 512], f32, tag="r_all")
        nc.scalar.copy(r_all[:], ps_rb[:])
        for t in range(NT):
            pp = ps_acc.tile([128, 512], f32, tag=f"o{t}")
            for ct in range(NT):
                nc.tensor.matmul(pp[:],
                                 w_t["p"][ct][:, t * 128:(t + 1) * 128],
                                 o_n[ct][:],
                                 start=(ct == 0), stop=(ct == 3))
            t1 = outsb.tile([128, 512], f32, tag="t1")
            nc.vector.tensor_tensor(t1[:], pp[:], r_all[:], op=ALU.mult)
            ot = outsb.tile([128, 512], f32, tag="ot")
            nc.vector.scalar_tensor_tensor(
                ot[:], t1[:], bp_t[t],
                x_sl[t][:, qch * 512:(qch + 1) * 512],
                op0=ALU.add, op1=ALU.add)
            nc.sync.dma_start(
                out.ap()[t * 128:(t + 1) * 128, qch * 512:(qch + 1) * 512],
                ot[:])


def _build():
    nc = bacc.Bacc("TRN2", target_bir_lowering=False, debug=False, num_devices=8)
    xbf_d = nc.dram_tensor("xbf", [C, N], bf16, kind="ExternalInput")
    xslab_d = nc.dram_tensor("xslab", [C, QS], f32, kind="ExternalInput")
    wall_d = nc.dram_tensor("wall", [128, 16 * 512], bf16, kind="ExternalInput")
    misc_d = nc.dram_tensor("misc", [128, 668], f32, kind="ExternalInput")
    out = nc.dram_tensor("out", [C, QS], f32, kind="ExternalOutput")
    with tile.TileContext(nc) as tc:
        with contextlib.ExitStack() as ctx:
            p = _make_pools(tc, ctx)
            _emit_body(nc, tc, p, xbf_d, xslab_d, wall_d, misc_d, out)
    nc.compile()
    return nc


def _build_timing(reps):
    """Same body repeated `reps` times in a hardware loop; inputs live in
    internal DRAM (no host transfer) so per-call wall time differences
    isolate on-device execution."""
    nc = bacc.Bacc("TRN2", target_bir_lowering=False, debug=False, num_devices=8)
    xbf_d = nc.dram_tensor("xbf", [C, N], bf16, kind="ExternalInput")
    xslab_d = nc.dram_tensor("xslab", [C, QS], f32, kind="ExternalInput")
    wall_d = nc.dram_tensor("wall", [128, 16 * 512], bf16, kind="ExternalInput")
    misc_d = nc.dram_tensor("misc", [128, 668], f32, kind="ExternalInput")
    out = nc.dram_tensor("out", [C, QS], f32, kind="ExternalOutput")
    with tile.TileContext(nc) as tc:
        with contextlib.ExitStack() as ctx:
            p = _make_pools(tc, ctx)
            if reps == 1:
                _emit_body(nc, tc, p, xbf_d, xslab_d, wall_d, misc_d, out)
            else:
                with tc.For_i(0, reps, 1):
                    _emit_body(nc, tc, p, xbf_d, xslab_d, wall_d, misc_d, out)
    nc.compile()
    return nc


_NC = None


def _get_nc():
    global _NC
    if _NC is None:
        _NC = _build()
    return _NC


def kernel(x, gn_w, gn_b, wq, bq, wk, bk, wv, bv, wp, bp):
    x = np.asarray(x, dtype=np.float32)
    B = x.shape[0]
    assert x.shape == (B, C, 64, 64)
    scale = float(C) ** -0.5

    wqT = np.ascontiguousarray((np.asarray(wq, np.float32) * scale).T
                               ).astype(ml_dtypes.bfloat16)
    wkT = np.ascontiguousarray(np.asarray(wk, np.float32).T).astype(ml_dtypes.bfloat16)
    wvT = np.ascontiguousarray(np.asarray(wv, np.float32).T).astype(ml_dtypes.bfloat16)
    wpT = np.ascontiguousarray(np.asarray(wp, np.float32).T).astype(ml_dtypes.bfloat16)
    wall = np.zeros((128, 16 * 512), ml_dtypes.bfloat16)
    for iw, wT in enumerate((wqT, wkT, wvT, wpT)):
        for t in range(NT):
            wall[:, (iw * 4 + t) * 512:(iw * 4 + t + 1) * 512] = \
                wT[t * 128:(t + 1) * 128, :]

    misc = np.zeros((128, 668), np.float32)
    bq_s = np.asarray(bq, np.float32) * scale
    bk_a = np.asarray(bk, np.float32)
    bp_a = np.asarray(bp, np.float32)
    gnw_a = np.asarray(gn_w, np.float32)
    gnb_a = np.asarray(gn_b, np.float32)
    for t in range(NT):
        sl = slice(t * 128, (t + 1) * 128)
        misc[:, 2 * t] = bq_s[sl]
        misc[:, 2 * t + 1] = bk_a[sl]
        misc[:, 8 + t] = bp_a[sl]
        misc[:, 12 + t] = gnw_a[sl]
        misc[:, 16 + t] = gnb_a[sl]
    sel8 = np.zeros((128, 8), np.float32)
    for pp in range(128):
        sel8[pp, pp // GP] = 1.0
    misc[:, 20:28] = sel8
    misc[0:8, 28:156] = sel8.T
    misc[0:1, 156:668] = np.asarray(bv, np.float32).reshape(1, C)

    xf = x.reshape(B, C, N)
    in_maps = []
    for core in range(8):
        b, slab = core // 4, core % 4
        xr = np.roll(xf[b], -QS * slab, axis=1)
        in_maps.append({
            "xbf": np.ascontiguousarray(xr).astype(ml_dtypes.bfloat16),
            "xslab": np.ascontiguousarray(xr[:, 0:QS]),
            "wall": wall, "misc": misc,
        })

    global _last_in_maps
    _last_in_maps = in_maps
    nc = _get_nc()
    res = bass_utils.run_bass_kernel_spmd(nc, in_maps, core_ids=list(range(8)))

    out = np.empty((B, C, N), np.float32)
    for core in range(8):
        b, slab = core // 4, core % 4
        out[b][:, QS * slab:QS * (slab + 1)] = res.results[core]["out"]
    return out.reshape(B, C, 64, 64)


if __name__ == "__main__":
    rng = np.random.default_rng(0)
    inputs = {
        "x": rng.standard_normal((2, C, 64, 64)).astype(np.float32),
        "gn_w": np.ones(C, np.float32),
        "gn_b": np.zeros(C, np.float32),
    }
    for nm in ("q", "k", "v", "p"):
        inputs[f"w{nm}"] = (rng.standard_normal((C, C)) * 0.02).astype(np.float32)
        inputs[f"b{nm}"] = np.zeros(C, np.float32)
    out = kernel(**inputs)
    print("ran:", out.shape, out.dtype)



# revision 3
# speedup vs baseline: 1.7238x; 1.7238x over previous
"""AttnBlock (GroupNorm -> QKV -> 4096x4096 single-head attention -> proj ->
residual) on 8 TRN2 NeuronCores.

Sharding: data-parallel over batch (B=2) x sequence-parallel over query
positions (4 slabs of 1024). Each core receives the full x[b] (rolled so its
query slab sits at columns 0:1024), computes GroupNorm + k/v for the whole
image (replicated within the 4-core batch group -> zero collectives), and
attention + projection + residual for its 1024 query columns only.

Precision: all heavy matmuls run fp8e4m3 with DoubleRow perf mode (2x128
contraction per instruction at 0.5 cyc/row), fp32 PSUM accumulation.
GroupNorm is folded into the moving operand: xn = scale*x + shift quantized
to fp8 once, weights are static fp8 (x32) from the host. GroupNorm stats are
computed on a stride-4 subsample (var estimator rel-err ~1.1%, far below the
fp8 path noise). Softmax denominators come from a DoubleRow matmul with a
constant-4.0 stationary; normalization is applied after the output
projection (o is drained to fp8 at a fixed 2^-8 scale, which keeps it in
fp8 range without needing the per-query reciprocal first).
"""
import sys
sys.path.insert(0, '/opt/trn_rl_repo')
import contextlib
import numpy as np
import ml_dtypes

import concourse.bass as bass
import concourse.tile as tile
from concourse import mybir, bacc
from concourse import bass_utils

f32 = mybir.dt.float32
bf16 = mybir.dt.bfloat16
fp8 = mybir.dt.float8e4
AF = mybir.ActivationFunctionType
ALU = mybir.AluOpType
DR = mybir.MatmulPerfMode.DoubleRow
F8 = ml_dtypes.float8_e4m3

C = 512          # channels
N = 4096         # positions (64*64)
G = 32           # groupnorm groups
GP = 16          # channels per group
NT = C // 128    # 4 channel partition-tiles
QS = 1024        # query slab per core
QC = 256         # query chunk in attention phase
EPS = 1e-6
SW = 32.0        # weight scale folded into all four fp8 weights
SEXP = 1.0 / (SW * SW * float(np.sqrt(C)))   # exp(psum * SEXP) = exp(score)
OSC = 2.0 ** -8  # o-psum -> fp8 drain scale
STRIDE = 4       # GN stats subsample stride
NSAMP = float(GP * (N // STRIDE))            # samples per group


def _emit_body(nc, tc, p, xbf_d, xsl_d, wall_d, misc_d, out):
    sb, scr, pq, outp, small = p["sb"], p["scr"], p["pq"], p["outp"], p["small"]
    ps_mm, ps_o, ps_sums, ps_sm = p["ps_mm"], p["ps_o"], p["ps_sums"], p["ps_sm"]

    # ---- constants / small tiles ---------------------------------
    ones_row = small.tile([1, 128], bf16, tag="ones_row")
    nc.vector.memset(ones_row[:], 1.0)
    ones4 = small.tile([128, 2, 32], fp8, tag="ones4")
    nc.vector.memset(ones4[:], 4.0)
    eps8 = small.tile([8, 1], f32, tag="eps8")
    nc.vector.memset(eps8[:], EPS)

    # ---- phase 0: DMA + GN stats + xn quantize -------------------
    xbf = sb.tile([128, NT, N], bf16, tag="xbf")
    nc.sync.dma_start(xbf[:, 0, :], xbf_d.ap()[0:128, :])
    wall = sb.tile([128, 4, NT, C], fp8, tag="wall")
    nc.sync.dma_start(wall[:], wall_d.ap())
    misc = sb.tile([128, 672], f32, tag="misc")
    nc.sync.dma_start(misc[:], misc_d.ap())
    for t in range(1, NT):
        nc.sync.dma_start(xbf[:, t, :], xbf_d.ap()[t * 128:(t + 1) * 128, :])
    x_sl = sb.tile([128, NT, QS], f32, tag="x_sl")
    for t in range(NT):
        nc.sync.dma_start(x_sl[:, t, :], xsl_d.ap()[t * 128:(t + 1) * 128, :])

    w_q, w_k, w_v, w_p = (wall[:, i, :, :] for i in range(4))
    bq_t = [misc[:, t:t + 1] for t in range(NT)]
    bk_t = [misc[:, 4 + t:5 + t] for t in range(NT)]
    bp_t = [misc[:, 8 + t:9 + t] for t in range(NT)]
    gnw_t = [misc[:, 12 + t:13 + t] for t in range(NT)]
    gnb_t = [misc[:, 16 + t:17 + t] for t in range(NT)]
    sel8 = misc[:, 20:28]
    sel8T = misc[0:8, 32:160]
    bv32 = misc[0:1, 160:672]

    # stats on a stride-4 subsample; sum on DVE, sumsq on ACT
    sum_c = small.tile([128, NT], f32, tag="sum_c")
    sq_c = small.tile([128, NT], f32, tag="sq_c")
    for t in range(NT):
        samp = xbf[:, t, :].rearrange("p (n s) -> p s n", s=STRIDE)[:, 0, :]
        nc.vector.reduce_sum(sum_c[:, t:t + 1], samp, axis=mybir.AxisListType.X)
        sqs = scr.tile([128, N // STRIDE], bf16, tag="sqs")
        nc.scalar.activation(sqs[:], samp, AF.Square, accum_out=sq_c[:, t:t + 1])

    # per-tile group stats -> scale/shift [128,1]
    scale_t, shift_t = [], []
    for t in range(NT):
        pair = small.tile([128, 2], f32, tag=f"pair{t}")
        nc.vector.tensor_copy(pair[:, 0:1], sum_c[:, t:t + 1])
        nc.vector.tensor_copy(pair[:, 1:2], sq_c[:, t:t + 1])
        ps_g = ps_sm.tile([8, 2], f32, tag="st")
        nc.tensor.matmul(ps_g[:], sel8, pair[:], start=True, stop=True)
        mstats = small.tile([8, 2], f32, tag=f"mst{t}")
        nc.scalar.mul(mstats[:], ps_g[:], 1.0 / NSAMP)
        var8 = small.tile([8, 1], f32, tag=f"var{t}")
        m2 = small.tile([8, 1], f32, tag=f"m2{t}")
        nc.vector.tensor_tensor(m2[:], mstats[:, 0:1], mstats[:, 0:1], op=ALU.mult)
        nc.vector.tensor_tensor(var8[:], mstats[:, 1:2], m2[:], op=ALU.subtract)
        lnv = small.tile([8, 1], f32, tag=f"lnv{t}")
        nc.scalar.activation(lnv[:], var8[:], AF.Ln, bias=eps8[:])
        grp2 = small.tile([8, 2], f32, tag=f"grp2{t}")
        nc.vector.tensor_copy(grp2[:, 0:1], mstats[:, 0:1])
        nc.scalar.activation(grp2[:, 1:2], lnv[:], AF.Exp, scale=-0.5)
        ps_bc = ps_sm.tile([128, 2], f32, tag="st")
        nc.tensor.matmul(ps_bc[:], sel8T, grp2[:], start=True, stop=True)
        sc = small.tile([128, 1], f32, tag=f"scale{t}")
        nc.vector.tensor_tensor(sc[:], gnw_t[t], ps_bc[:, 1:2], op=ALU.mult)
        nsc = small.tile([128, 1], f32, tag=f"nscale{t}")
        nc.vector.tensor_scalar_mul(nsc[:], sc[:], -1.0)
        sh = small.tile([128, 1], f32, tag=f"shift{t}")
        nc.vector.scalar_tensor_tensor(sh[:], ps_bc[:, 0:1], nsc[:], gnb_t[t],
                                       op0=ALU.mult, op1=ALU.add)
        scale_t.append(sc)
        shift_t.append(sh)

    # xn = fp8(scale*x + shift), split Pool / ACT / DVE
    xn = sb.tile([128, NT, N], fp8, tag="xn")
    for t in range(NT):
        h = N // 2
        nc.gpsimd.tensor_scalar(xn[:, t, 0:h], xbf[:, t, 0:h],
                                scale_t[t][:], shift_t[t][:],
                                op0=ALU.mult, op1=ALU.add)
        if t % 2 == 0:
            nc.scalar.activation(xn[:, t, h:N], xbf[:, t, h:N], AF.Identity,
                                 bias=shift_t[t][:], scale=scale_t[t][:])
        else:
            nc.vector.tensor_scalar(xn[:, t, h:N], xbf[:, t, h:N],
                                    scale_t[t][:], shift_t[t][:],
                                    op0=ALU.mult, op1=ALU.add)

    # fold bp into the residual slab (Pool, off critical path)
    for t in range(NT):
        nc.gpsimd.tensor_scalar_add(x_sl[:, t, :], x_sl[:, t, :], bp_t[t])

    # broadcast 32*bv across partitions: [1,512] -> [128, 2, 512] (doubled
    # so the v drain's in1 matches a [128, 2, 512] psum view)
    bv_bf = small.tile([1, C], bf16, tag="bv_bf")
    nc.vector.tensor_copy(bv_bf[:], bv32)
    ps_bv = ps_mm.tile([128, 1024], f32, tag="mm")
    nc.tensor.matmul(ps_bv[:, 0:512], ones_row[:], bv_bf[:], start=True, stop=True)
    bv_bc2 = sb.tile([128, 2, C], f32, tag="bv_bc2")
    nc.scalar.copy(bv_bc2[:, 0, :], ps_bv[:, 0:512])
    nc.scalar.copy(bv_bc2[:, 1, :], ps_bv[:, 0:512])

    # ---- phase 1: q / k / v projections (all DR fp8) -------------
    # q: [128, t, 1024]  = 32*(q + bq)
    q_f8 = sb.tile([128, NT, QS], fp8, tag="q_f8")
    for t in range(NT):
        psq = ps_mm.tile([128, 1024], f32, tag="mm")
        for qc2 in range(4):
            for a in range(2):
                nc.tensor.matmul(psq[:, qc2 * 256:(qc2 + 1) * 256],
                                 w_q[:, 2 * a:2 * a + 2, t * 128:(t + 1) * 128],
                                 xn[:, 2 * a:2 * a + 2, qc2 * 256:(qc2 + 1) * 256],
                                 start=(a == 0 and qc2 % 2 == 0), stop=(a == 1),
                                 perf_mode=DR)
        nc.scalar.activation(q_f8[:, t, :], psq[:], AF.Identity, bias=bq_t[t])

    # k: [128, t, 4096] = 32*(k + bk)
    k_f8 = sb.tile([128, NT, N], fp8, tag="k_f8")
    for t in range(NT):
        for ch in range(4):
            psk = ps_mm.tile([128, 1024], f32, tag="mm")
            for qc2 in range(4):
                for a in range(2):
                    col = ch * 1024 + qc2 * 256
                    nc.tensor.matmul(psk[:, qc2 * 256:(qc2 + 1) * 256],
                                     w_k[:, 2 * a:2 * a + 2, t * 128:(t + 1) * 128],
                                     xn[:, 2 * a:2 * a + 2, col:col + 256],
                                     start=(a == 0 and qc2 % 2 == 0), stop=(a == 1),
                                     perf_mode=DR)
            nc.vector.tensor_scalar_add(k_f8[:, t, ch * 1024:(ch + 1) * 1024],
                                        psk[:], bk_t[t])

    # vT: [128 keys, nt, 512] = 32*(v + bv)
    vt_f8 = sb.tile([128, N // 128, C], fp8, tag="vt_f8")
    for np_ in range(16):
        psv = ps_mm.tile([128, 2, C], f32, tag="mm")
        for j in range(2):
            nt = 2 * np_ + j
            for co in range(2):
                for a in range(2):
                    nc.tensor.matmul(psv[:, j, co * 256:(co + 1) * 256],
                                     xn[:, 2 * a:2 * a + 2, nt * 128:(nt + 1) * 128],
                                     w_v[:, 2 * a:2 * a + 2, co * 256:(co + 1) * 256],
                                     start=(co == 0 and a == 0), stop=(a == 1),
                                     perf_mode=DR)
        nc.vector.tensor_tensor(vt_f8[:, 2 * np_:2 * np_ + 2, :], psv[:],
                                bv_bc2[:], op=ALU.add)

    # ---- phase 2: attention + proj per 256-query chunk -----------
    for qch in range(QS // QC):
        q0 = qch * QC
        ob = [ps_o.tile([128, 2, QC], f32, name=f"ob{half}", tag=f"ob{half}")
              for half in range(2)]
        sums_ps = ps_sums.tile([32, 512], f32, tag="sums")
        for qd in range(8):
            stq = ps_mm.tile([128, 4, QC], f32, tag="mm")
            for j in range(4):
                kt = qd * 4 + j
                for a in range(2):
                    nc.tensor.matmul(stq[:, j, :],
                                     k_f8[:, 2 * a:2 * a + 2, kt * 128:(kt + 1) * 128],
                                     q_f8[:, 2 * a:2 * a + 2, q0:q0 + QC],
                                     start=(j % 2 == 0 and a == 0), stop=(a == 1),
                                     perf_mode=DR)
            p_q = pq.tile([128, 4, QC], fp8, tag="p")
            nc.scalar.activation(p_q[:], stq[:], AF.Exp, scale=SEXP)
            for a2 in range(2):
                kt0 = qd * 4 + 2 * a2
                first = (qd == 0 and a2 == 0)
                last = (qd == 7 and a2 == 1)
                for ct in range(4):
                    nc.tensor.matmul(ob[ct // 2][:, ct % 2, :],
                                     vt_f8[:, kt0:kt0 + 2, ct * 128:(ct + 1) * 128],
                                     p_q[:, 2 * a2:2 * a2 + 2, :],
                                     start=(first and ct % 2 == 0), stop=last,
                                     perf_mode=DR)
                nc.tensor.matmul(sums_ps[:, 0:QC], ones4[:],
                                 p_q[:, 2 * a2:2 * a2 + 2, :],
                                 start=first, stop=last, perf_mode=DR)
        # softmax denominators -> r_bc2 [128, 2, QC] f32
        r_sb = small.tile([1, QC], f32, tag="r_sb")
        nc.vector.reciprocal(r_sb[:], sums_ps[0:1, 0:QC])
        r_bf = small.tile([1, QC], bf16, tag="r_bf")
        nc.vector.tensor_copy(r_bf[:], r_sb[:])
        ps_r = ps_mm.tile([128, 1024], f32, tag="mm")
        nc.tensor.matmul(ps_r[:, 0:QC], ones_row[:], r_bf[:], start=True, stop=True)
        r_bc2 = small.tile([128, 2, QC], f32, tag="r_bc2")
        nc.scalar.copy(r_bc2[:, 0, :], ps_r[:, 0:QC])
        nc.scalar.copy(r_bc2[:, 1, :], ps_r[:, 0:QC])
        # o -> fp8 at fixed 2^-8 scale
        o_f8 = scr.tile([128, 4, QC], fp8, tag="o_f8")
        nc.scalar.mul(o_f8[:, 0:2, :], ob[0][:], OSC)
        nc.scalar.mul(o_f8[:, 2:4, :], ob[1][:], OSC)
        # proj (reuses the o psum banks) + normalize + residual
        for half in range(2):
            ppb = ps_o.tile([128, 2, QC], f32, tag=f"ob{half}")
            for tl in range(2):
                t = 2 * half + tl
                for a in range(2):
                    nc.tensor.matmul(ppb[:, tl, :],
                                     w_p[:, 2 * a:2 * a + 2, t * 128:(t + 1) * 128],
                                     o_f8[:, 2 * a:2 * a + 2, :],
                                     start=(tl == 0 and a == 0), stop=(a == 1),
                                     perf_mode=DR)
            tt = outp.tile([128, 2, QC], f32, tag="tt")
            nc.vector.tensor_tensor(tt[:], ppb[:], r_bc2[:], op=ALU.mult)
            outb = outp.tile([128, 2, QC], f32, tag="outb")
            nc.gpsimd.tensor_tensor(outb[:], tt[:],
                                    x_sl[:, 2 * half:2 * half + 2, q0:q0 + QC],
                                    op=ALU.add)
            for tl in range(2):
                nc.sync.dma_start(
                    out.ap()[(2 * half + tl) * 128:(2 * half + tl + 1) * 128,
                             q0:q0 + QC],
                    outb[:, tl, :])


def _make_pools(tc, ctx):
    p = {}
    p["sb"] = ctx.enter_context(tc.tile_pool(name="sb", bufs=1))
    p["scr"] = ctx.enter_context(tc.tile_pool(name="scr", bufs=2))
    p["pq"] = ctx.enter_context(tc.tile_pool(name="pq", bufs=2))
    p["outp"] = ctx.enter_context(tc.tile_pool(name="outp", bufs=2))
    p["small"] = ctx.enter_context(tc.tile_pool(name="small", bufs=1))
    p["ps_mm"] = ctx.enter_context(tc.tile_pool(name="ps_mm", bufs=2, space="PSUM"))
    p["ps_o"] = ctx.enter_context(tc.tile_pool(name="ps_o", bufs=1, space="PSUM"))
    p["ps_sums"] = ctx.enter_context(tc.tile_pool(name="ps_sums", bufs=1, space="PSUM"))
    p["ps_sm"] = ctx.enter_context(tc.tile_pool(name="ps_sm", bufs=1, space="PSUM"))
    return p


def _build():
    nc = bacc.Bacc("TRN2", target_bir_lowering=False, debug=False, num_devices=8)
    xbf_d = nc.dram_tensor("xbf", [C, N], bf16, kind="ExternalInput")
    xsl_d = nc.dram_tensor("xsl", [C, QS], f32, kind="ExternalInput")
    wall_d = nc.dram_tensor("wall", [128, 4 * NT * C], fp8, kind="ExternalInput")
    misc_d = nc.dram_tensor("misc", [128, 672], f32, kind="ExternalInput")
    out = nc.dram_tensor("out", [C, QS], f32, kind="ExternalOutput")
    with tile.TileContext(nc) as tc:
        with contextlib.ExitStack() as ctx:
            p = _make_pools(tc, ctx)
            _emit_body(nc, tc, p, xbf_d, xsl_d, wall_d, misc_d, out)
    nc.compile()
    return nc


_NC = None


def _get_nc():
    global _NC
    if _NC is None:
        _NC = _build()
    return _NC


def kernel(x, gn_w, gn_b, wq, bq, wk, bk, wv, bv, wp, bp):
    x = np.asarray(x, dtype=np.float32)
    B = x.shape[0]
    assert x.shape == (B, C, 64, 64)

    # weights: [p, cin_sub, cout] = 32 * W[cout, cin_sub*128+p], fp8
    wall = np.zeros((128, 4, NT, C), F8)
    for iw, w in enumerate((wq, wk, wv, wp)):
        wt = (np.asarray(w, np.float32) * SW).T   # [cin, cout]
        wall[:, iw, :, :] = wt.reshape(NT, 128, C).transpose(1, 0, 2).astype(F8)
    wall = wall.reshape(128, 4 * NT * C)

    misc = np.zeros((128, 672), np.float32)
    for t in range(NT):
        sl = slice(t * 128, (t + 1) * 128)
        misc[:, t] = np.asarray(bq, np.float32)[sl] * SW
        misc[:, 4 + t] = np.asarray(bk, np.float32)[sl] * SW
        misc[:, 8 + t] = np.asarray(bp, np.float32)[sl]
        misc[:, 12 + t] = np.asarray(gn_w, np.float32)[sl]
        misc[:, 16 + t] = np.asarray(gn_b, np.float32)[sl]
    sel8 = np.zeros((128, 8), np.float32)
    for pp_ in range(128):
        sel8[pp_, pp_ // GP] = 1.0
    misc[:, 20:28] = sel8
    misc[0:8, 32:160] = sel8.T
    misc[0:1, 160:672] = np.asarray(bv, np.float32).reshape(1, C) * SW

    xf = x.reshape(B, C, N)
    in_maps = []
    for core in range(8):
        b, slab = core // 4, core % 4
        xr = np.roll(xf[b], -QS * slab, axis=1)
        in_maps.append({
            "xbf": np.ascontiguousarray(xr).astype(ml_dtypes.bfloat16),
            "xsl": np.ascontiguousarray(xr[:, 0:QS]),
            "wall": wall, "misc": misc,
        })

    nc = _get_nc()
    res = bass_utils.run_bass_kernel_spmd(nc, in_maps, core_ids=list(range(8)))

    out = np.empty((B, C, N), np.float32)
    for core in range(8):
        b, slab = core // 4, core % 4
        out[b][:, QS * slab:QS * (slab + 1)] = res.results[core]["out"]
    return out.reshape(B, C, 64, 64)


if __name__ == "__main__":
    rng = np.random.default_rng(0)
    inputs = {
        "x": rng.standard_normal((2, C, 64, 64)).astype(np.float32),
        "gn_w": np.ones(C, np.float32),
        "gn_b": np.zeros(C, np.float32),
    }
    for nm in ("q", "k", "v", "p"):
        inputs[f"w{nm}"] = (rng.standard_normal((C, C)) * 0.02).astype(np.float32)
        inputs[f"b{nm}"] = np.zeros(C, np.float32)
    out = kernel(**inputs)
    print("ran:", out.shape, out.dtype)


# revision 9
# speedup vs baseline: 2.0559x; 1.1926x over previous
"""AttnBlock (GroupNorm -> QKV -> 4096x4096 single-head attention -> proj ->
residual) on 8 TRN2 NeuronCores.

Sharding: data-parallel over batch (B=2) x sequence-parallel over query
positions (4 slabs of 1024). Each core receives the full x[b] (rolled so its
query slab sits at columns 0:1024), computes GroupNorm + v for the whole
image (replicated within the 4-core batch group -> zero collectives), and
attention + projection + residual for its 1024 query columns only.

Precision/structure highlights:
- All heavy matmuls are fp8e4m3 DoubleRow (2x128 contraction / instr at
  0.5 cyc/row), fp32 PSUM.
- k is never materialized: score = (Wk^T q)^T . h, so we compute
  u = Wk^T q_f8 (C x 1024) and use the fp8 x itself as the score
  stationary. The bk term is per-query constant -> softmax invariant ->
  dropped.
- GroupNorm folds: scale -> folded into wq/wv weight tiles on device
  (per-partition) and into the u drain; shift -> tiny N=1 bias-chain
  matmuls (bq2 = 32(Wq shift + bq)) plus a per-channel residual constant
  out += Wp (Wv shift + bv) handled via x_sl. GN stats use a stride-8
  subsample (var estimator err ~1.6%, far below fp8 noise).
- Softmax denominators via a DoubleRow matmul with constant-4.0
  stationary; the o drain fuses the normalization (tensor_tensor with the
  broadcast reciprocal).
"""
import sys
sys.path.insert(0, '/opt/trn_rl_repo')
import contextlib
import numpy as np
import ml_dtypes

import concourse.bass as bass
import concourse.tile as tile
from concourse import mybir, bacc
from concourse import bass_utils

f32 = mybir.dt.float32
bf16 = mybir.dt.bfloat16
fp8 = mybir.dt.float8e4
AF = mybir.ActivationFunctionType
ALU = mybir.AluOpType
DR = mybir.MatmulPerfMode.DoubleRow
F8 = ml_dtypes.float8_e4m3

C = 512          # channels
N = 4096         # positions (64*64)
G = 32           # groupnorm groups
GP = 16          # channels per group
NT = C // 128    # 4 channel partition-tiles
QS = 1024        # query slab per core
QC = 256         # query chunk in attention phase
EPS = 1e-6
SW = 32.0        # weight scale folded into all four fp8 weights
SEXP = 1.0 / (16.0 * float(np.sqrt(C)))  # score_psum = 16 * qk/1 -> *SEXP
OSC = 2.0 ** -8  # final pp drain scale (pp = 256 * wp.o_norm)
STRIDE = 8       # GN stats subsample stride
NSAMP = float(GP * (N // STRIDE))        # samples per group


def _emit_body(nc, tc, p, x8_d, xsl_d, wall_d, misc_d, out):
    sb, scr, pq, outp, small = p["sb"], p["scr"], p["pq"], p["outp"], p["small"]
    ps_mm, ps_o, ps_sums, ps_sm = p["ps_mm"], p["ps_o"], p["ps_sums"], p["ps_sm"]

    ones_row = small.tile([1, 128], bf16, tag="ones_row")
    nc.vector.memset(ones_row[:], 1.0)
    ones4 = small.tile([128, 2, 32], fp8, tag="ones4")
    nc.vector.memset(ones4[:], 4.0)
    eps8 = small.tile([8, 1], f32, tag="eps8")
    nc.vector.memset(eps8[:], EPS)

    # ---- phase 0: DMA + GN stats ---------------------------------
    x8 = sb.tile([128, NT, N], fp8, tag="x8")
    nc.sync.dma_start(x8[:, 0, :], x8_d.ap()[0:128, :])
    wall = sb.tile([128, 4, NT, C], fp8, tag="wall")
    nc.sync.dma_start(wall[:], wall_d.ap())
    misc = sb.tile([128, 672], f32, tag="misc")
    nc.sync.dma_start(misc[:], misc_d.ap())
    for t in range(1, NT):
        nc.sync.dma_start(x8[:, t, :], x8_d.ap()[t * 128:(t + 1) * 128, :])

    w_q, w_kT, w_v, w_p = (wall[:, i, :, :] for i in range(4))
    bq32 = [misc[:, t:t + 1] for t in range(NT)]          # 32*bq
    bv32 = [misc[:, 4 + t:5 + t] for t in range(NT)]      # 32*bv (cout tiles)
    bp_t = [misc[:, 8 + t:9 + t] for t in range(NT)]
    gnw_t = [misc[:, 12 + t:13 + t] for t in range(NT)]
    gnb_t = [misc[:, 16 + t:17 + t] for t in range(NT)]
    sel8p = misc[:, 20:148]        # [128,128] group selector, zero-padded
    sel8T = misc[0:8, 160:288]

    # stats on stride-8 subsample: sum on DVE, sumsq on ACT
    stats2 = small.tile([128, 8], f32, tag="stats2")      # cols 0:4 sum, 4:8 sq
    for t in range(NT):
        samp = x8[:, t, :].rearrange("p (n s) -> p s n", s=STRIDE)[:, 0, :]
        nc.vector.reduce_sum(stats2[:, t:t + 1], samp, axis=mybir.AxisListType.X)
        sqs = scr.tile([128, N // STRIDE], bf16, tag="sqs")
        nc.scalar.activation(sqs[:], samp, AF.Square,
                             accum_out=stats2[:, 4 + t:5 + t])

    # batched GN finalize: one Ln + one Exp for all 32 groups
    ps_all = ps_sm.tile([128, 512], f32, tag="sm")    # one bank for all
    # sel8p is zero-padded to 128 stationary columns so this first matmul
    # covers all 128 partitions: its start=True pending-zeroes the whole
    # bank for every later start=False matmul below.
    ps_g = ps_all[:, 0:8]
    nc.tensor.matmul(ps_g, sel8p, stats2[:], start=True, stop=True)
    mst = small.tile([8, 8], f32, tag="mst")
    nc.scalar.mul(mst[:], ps_g[0:8, :], 1.0 / NSAMP)           # mean t | ex2 t
    var4 = small.tile([8, 4], f32, tag="var4")
    m2 = small.tile([8, 4], f32, tag="m2")
    nc.vector.tensor_tensor(m2[:], mst[:, 0:4], mst[:, 0:4], op=ALU.mult)
    nc.vector.tensor_tensor(var4[:], mst[:, 4:8], m2[:], op=ALU.subtract)
    lnv = small.tile([8, 4], f32, tag="lnv")
    nc.scalar.activation(lnv[:], var4[:], AF.Ln, bias=eps8[:])
    grp2 = small.tile([8, 2, NT], f32, tag="grp2")        # mean row | rstd row
    nc.vector.tensor_copy(grp2[:, 0, :], mst[:, 0:4])
    nc.scalar.activation(grp2[:, 1, :], lnv[:], AF.Exp, scale=-0.5)
    # NOTE: ps_g's start=True zeroed this whole PSUM bank; every later
    # matmul into the ps_sm bank must use start=False (pending-zero) so it
    # does not clobber earlier results that are still being read.
    ps_bc = ps_all[:, 8:16]                               # mean t | rstd t
    nc.tensor.matmul(ps_bc, sel8T, grp2[:].rearrange("p a b -> p (a b)"),
                     start=False, stop=True)
    # scale/shift per tile + derived columns
    scale_t, sc6_t = [], []
    sh8 = small.tile([128, NT], fp8, tag="sh8")           # 128*shift, fp8
    for t in range(NT):
        sc = small.tile([128, 1], f32, tag=f"scale{t}")
        nc.vector.tensor_tensor(sc[:], gnw_t[t], ps_bc[:, 4 + t:5 + t],
                                op=ALU.mult)
        sc6 = small.tile([128, 1], f32, tag=f"sc6{t}")
        nc.vector.tensor_scalar_mul(sc6[:], sc[:], 2.0 ** -6)
        nsc = small.tile([128, 1], f32, tag=f"nscale{t}")
        nc.vector.tensor_scalar_mul(nsc[:], sc[:], -1.0)
        sh = small.tile([128, 1], f32, tag=f"shift{t}")
        nc.vector.scalar_tensor_tensor(sh[:], ps_bc[:, t:t + 1], nsc[:],
                                       gnb_t[t], op0=ALU.mult, op1=ALU.add)
        nc.vector.tensor_scalar_mul(sh8[:, t:t + 1], sh[:], 128.0)
        scale_t.append(sc)
        sc6_t.append(sc6)

    # bias chains (N=1 fp8 matmuls, before the in-place weight folds):
    # bq2 = 32(Wq shift + bq);  vc = 2^5 (Wv shift + bv);
    # badd = Wp vc / 2^10  (-> residual constant Wp(Wv shift + bv))
    ps_bq = ps_all[:, 16:20]
    ps_vc = ps_all[:, 20:24]
    for t in range(NT):
        for s in range(NT):
            nc.tensor.matmul(ps_bq[:, t:t + 1],
                             w_q[:, s, t * 128:(t + 1) * 128], sh8[:, s:s + 1],
                             start=False, stop=(s == 3))
            nc.tensor.matmul(ps_vc[:, t:t + 1],
                             w_v[:, s, t * 128:(t + 1) * 128], sh8[:, s:s + 1],
                             start=False, stop=(s == 3))
    bq2 = small.tile([128, 4], f32, tag="bq2")
    for t in range(NT):
        nc.vector.tensor_scalar(bq2[:, t:t + 1], ps_bq[:, t:t + 1],
                                2.0 ** -7, bq32[t], op0=ALU.mult, op1=ALU.add)
    vc8 = small.tile([128, 4], fp8, tag="vc8")
    for t in range(NT):
        nc.scalar.activation(vc8[:, t:t + 1], ps_vc[:, t:t + 1], AF.Identity,
                             scale=2.0 ** -7, bias=bv32[t])
    ps_t2 = ps_all[:, 24:28]
    for t in range(NT):
        for s in range(NT):
            nc.tensor.matmul(ps_t2[:, t:t + 1],
                             w_p[:, s, t * 128:(t + 1) * 128], vc8[:, s:s + 1],
                             start=False, stop=(s == 3))
    badd = small.tile([128, 4], f32, tag="badd")
    for t in range(NT):
        nc.vector.tensor_scalar(badd[:, t:t + 1], ps_t2[:, t:t + 1],
                                2.0 ** -10, bp_t[t], op0=ALU.mult, op1=ALU.add)

    # fold GN scale into wq / wv (per-partition, in place, Pool)
    for s in range(NT):
        nc.gpsimd.tensor_scalar_mul(w_q[:, s, :], w_q[:, s, :], scale_t[s][:])
        nc.gpsimd.tensor_scalar_mul(w_v[:, s, :], w_v[:, s, :], scale_t[s][:])

    # residual slab + per-channel constant (bp + Wp(Wv shift + bv))
    x_sl = sb.tile([128, NT, QS], f32, tag="x_sl")
    for t in range(NT):
        nc.sync.dma_start(x_sl[:, t, :], xsl_d.ap()[t * 128:(t + 1) * 128, :])
    for t in range(NT):
        nc.gpsimd.tensor_scalar_add(x_sl[:, t, :], x_sl[:, t, :],
                                    badd[:, t:t + 1])

    # ---- phase 1: q, u = Wk^T q, vT ------------------------------
    q_f8 = sb.tile([128, NT, QS], fp8, tag="q_f8")
    for t in range(NT):
        psq = ps_mm.tile([128, 1024], f32, tag="mm")
        for qc2 in range(4):
            for a in range(2):
                nc.tensor.matmul(psq[:, qc2 * 256:(qc2 + 1) * 256],
                                 w_q[:, 2 * a:2 * a + 2, t * 128:(t + 1) * 128],
                                 x8[:, 2 * a:2 * a + 2, qc2 * 256:(qc2 + 1) * 256],
                                 start=(a == 0 and qc2 % 2 == 0), stop=(a == 1),
                                 perf_mode=DR)
        nc.scalar.activation(q_f8[:, t, :], psq[:], AF.Identity,
                             bias=bq2[:, t:t + 1])

    u_f8 = sb.tile([128, NT, QS], fp8, tag="u_f8")
    for t in range(NT):
        psu = ps_mm.tile([128, 1024], f32, tag="mm")
        for qc2 in range(4):
            for a in range(2):
                nc.tensor.matmul(psu[:, qc2 * 256:(qc2 + 1) * 256],
                                 w_kT[:, 2 * a:2 * a + 2, t * 128:(t + 1) * 128],
                                 q_f8[:, 2 * a:2 * a + 2, qc2 * 256:(qc2 + 1) * 256],
                                 start=(a == 0 and qc2 % 2 == 0), stop=(a == 1),
                                 perf_mode=DR)
        nc.scalar.mul(u_f8[:, t, :], psu[:], sc6_t[t][:])

    vt_f8 = sb.tile([128, N // 128, C], fp8, tag="vt_f8")
    for np_ in range(16):
        psv = ps_mm.tile([128, 2, C], f32, tag="mm")
        for j in range(2):
            nt = 2 * np_ + j
            for co in range(2):
                for a in range(2):
                    nc.tensor.matmul(psv[:, j, co * 256:(co + 1) * 256],
                                     x8[:, 2 * a:2 * a + 2, nt * 128:(nt + 1) * 128],
                                     w_v[:, 2 * a:2 * a + 2, co * 256:(co + 1) * 256],
                                     start=(co == 0 and a == 0), stop=(a == 1),
                                     perf_mode=DR)
        if np_ % 2 == 0:
            nc.vector.tensor_copy(vt_f8[:, 2 * np_:2 * np_ + 2, :], psv[:])
        else:
            nc.scalar.copy(vt_f8[:, 2 * np_:2 * np_ + 2, :], psv[:])

    # ---- phase 2: attention + proj per 256-query chunk -----------
    for qch in range(QS // QC):
        q0 = qch * QC
        ob = [ps_o.tile([128, 2, QC], f32, name=f"ob{half}", tag=f"ob{half}")
              for half in range(2)]
        sums_ps = ps_sums.tile([32, 512], f32, tag="sums")
        # software-pipelined quads: o/sums for quad qd-1 are emitted after
        # scores+exp of quad qd, so PE never waits on the current exp.
        pend = None

        def flush(pp_q, qd):
            for a2 in range(2):
                kt0 = qd * 4 + 2 * a2
                first = (qd == 0 and a2 == 0)
                last = (qd == 7 and a2 == 1)
                for ct in range(4):
                    nc.tensor.matmul(ob[ct // 2][:, ct % 2, :],
                                     vt_f8[:, kt0:kt0 + 2, ct * 128:(ct + 1) * 128],
                                     pp_q[:, 2 * a2:2 * a2 + 2, :],
                                     start=(first and ct % 2 == 0), stop=last,
                                     perf_mode=DR)
                nc.tensor.matmul(sums_ps[:, 0:QC], ones4[:],
                                 pp_q[:, 2 * a2:2 * a2 + 2, :],
                                 start=first, stop=last, perf_mode=DR)

        for qd in range(8):
            stq = ps_mm.tile([128, 4, QC], f32, tag="mm")
            for j in range(4):
                kt = qd * 4 + j
                for a in range(2):
                    nc.tensor.matmul(stq[:, j, :],
                                     x8[:, 2 * a:2 * a + 2, kt * 128:(kt + 1) * 128],
                                     u_f8[:, 2 * a:2 * a + 2, q0:q0 + QC],
                                     start=(j % 2 == 0 and a == 0), stop=(a == 1),
                                     perf_mode=DR)
            p_q = pq.tile([128, 4, QC], fp8, tag="p")
            nc.scalar.activation(p_q[:], stq[:], AF.Exp, scale=SEXP)
            if pend is not None:
                flush(*pend)
            pend = (p_q, qd)
        flush(*pend)

        r_sb = small.tile([1, QC], f32, tag="r_sb")
        nc.vector.reciprocal(r_sb[:], sums_ps[0:1, 0:QC])
        r_bf = small.tile([1, QC], bf16, tag="r_bf")
        nc.vector.tensor_copy(r_bf[:], r_sb[:])
        ps_r = ps_mm.tile([128, 1024], f32, tag="mm")
        nc.tensor.matmul(ps_r[:, 0:QC], ones_row[:], r_bf[:], start=True, stop=True)
        r_bc2 = small.tile([128, 2, QC], f32, tag="r_bc2")
        nc.scalar.copy(r_bc2[:, 0, :], ps_r[:, 0:QC])
        nc.scalar.copy(r_bc2[:, 1, :], ps_r[:, 0:QC])
        # o drain fuses softmax normalization: o_f8 = ob * r (= 8 * o_norm)
        o_f8 = scr.tile([128, 4, QC], fp8, tag="o_f8")
        nc.vector.tensor_tensor(o_f8[:, 0:2, :], ob[0][:], r_bc2[:], op=ALU.mult)
        nc.vector.tensor_tensor(o_f8[:, 2:4, :], ob[1][:], r_bc2[:], op=ALU.mult)
        for half in range(2):
            ppb = ps_o.tile([128, 2, QC], f32, name=f"pp{half}", tag=f"ob{half}")
            for tl in range(2):
                t = 2 * half + tl
                for a in range(2):
                    nc.tensor.matmul(ppb[:, tl, :],
                                     w_p[:, 2 * a:2 * a + 2, t * 128:(t + 1) * 128],
                                     o_f8[:, 2 * a:2 * a + 2, :],
                                     start=(tl == 0 and a == 0), stop=(a == 1),
                                     perf_mode=DR)
            outb = outp.tile([128, 2, QC], f32, tag="outb")
            nc.vector.scalar_tensor_tensor(
                outb[:], ppb[:], OSC,
                x_sl[:, 2 * half:2 * half + 2, q0:q0 + QC],
                op0=ALU.mult, op1=ALU.add)
            for tl in range(2):
                nc.sync.dma_start(
                    out.ap()[(2 * half + tl) * 128:(2 * half + tl + 1) * 128,
                             q0:q0 + QC],
                    outb[:, tl, :])


def _make_pools(tc, ctx):
    p = {}
    p["sb"] = ctx.enter_context(tc.tile_pool(name="sb", bufs=1))
    p["scr"] = ctx.enter_context(tc.tile_pool(name="scr", bufs=2))
    p["pq"] = ctx.enter_context(tc.tile_pool(name="pq", bufs=2))
    p["outp"] = ctx.enter_context(tc.tile_pool(name="outp", bufs=2))
    p["small"] = ctx.enter_context(tc.tile_pool(name="small", bufs=1))
    p["ps_mm"] = ctx.enter_context(tc.tile_pool(name="ps_mm", bufs=2, space="PSUM"))
    p["ps_o"] = ctx.enter_context(tc.tile_pool(name="ps_o", bufs=1, space="PSUM"))
    p["ps_sums"] = ctx.enter_context(tc.tile_pool(name="ps_sums", bufs=1, space="PSUM"))
    p["ps_sm"] = ctx.enter_context(tc.tile_pool(name="ps_sm", bufs=1, space="PSUM"))
    return p


def _build():
    nc = bacc.Bacc("TRN2", target_bir_lowering=False, debug=False, num_devices=8)
    x8_d = nc.dram_tensor("x8", [C, N], fp8, kind="ExternalInput")
    xsl_d = nc.dram_tensor("xsl", [C, QS], f32, kind="ExternalInput")
    wall_d = nc.dram_tensor("wall", [128, 4 * NT * C], fp8, kind="ExternalInput")
    misc_d = nc.dram_tensor("misc", [128, 672], f32, kind="ExternalInput")
    out = nc.dram_tensor("out", [C, QS], f32, kind="ExternalOutput")
    with tile.TileContext(nc) as tc:
        with contextlib.ExitStack() as ctx:
            p = _make_pools(tc, ctx)
            _emit_body(nc, tc, p, x8_d, xsl_d, wall_d, misc_d, out)
    nc.compile()
    return nc


_NC = None


def _get_nc():
    global _NC
    if _NC is None:
        _NC = _build()
    return _NC


def kernel(x, gn_w, gn_b, wq, bq, wk, bk, wv, bv, wp, bp):
    x = np.asarray(x, dtype=np.float32)
    B = x.shape[0]
    assert x.shape == (B, C, 64, 64)

    # weight slots: q/v/p transposed ([p, cin_sub, cout] = 32*W[cout, .]),
    # k slot NOT transposed ([p, cout_sub, cin] = 32*W[cout_sub*128+p, cin])
    # since it is the stationary of u = Wk^T q.
    wall = np.zeros((128, 4, NT, C), F8)
    for iw, (w, tr) in enumerate(((wq, True), (wk, False), (wv, True),
                                 (wp, True))):
        wm = np.asarray(w, np.float32) * SW
        if tr:
            wm = wm.T
        wall[:, iw, :, :] = wm.reshape(NT, 128, C).transpose(1, 0, 2).astype(F8)
    wall = wall.reshape(128, 4 * NT * C)

    misc = np.zeros((128, 672), np.float32)
    for t in range(NT):
        sl = slice(t * 128, (t + 1) * 128)
        misc[:, t] = np.asarray(bq, np.float32)[sl] * SW
        misc[:, 4 + t] = np.asarray(bv, np.float32)[sl] * SW
        misc[:, 8 + t] = np.asarray(bp, np.float32)[sl]
        misc[:, 12 + t] = np.asarray(gn_w, np.float32)[sl]
        misc[:, 16 + t] = np.asarray(gn_b, np.float32)[sl]
    sel8 = np.zeros((128, 8), np.float32)
    for pp_ in range(128):
        sel8[pp_, pp_ // GP] = 1.0
    misc[:, 20:148] = np.concatenate(
        [sel8, np.zeros((128, 120), np.float32)], axis=1)
    misc[0:8, 160:288] = sel8.T

    xf = x.reshape(B, C, N)
    in_maps = []
    for core in range(8):
        b, slab = core // 4, core % 4
        xr = np.roll(xf[b], -QS * slab, axis=1)
        in_maps.append({
            "x8": np.ascontiguousarray(xr).astype(F8),
            "xsl": np.ascontiguousarray(xr[:, 0:QS]),
            "wall": wall, "misc": misc,
        })

    nc = _get_nc()
    res = bass_utils.run_bass_kernel_spmd(nc, in_maps, core_ids=list(range(8)))

    out = np.empty((B, C, N), np.float32)
    for core in range(8):
        b, slab = core // 4, core % 4
        out[b][:, QS * slab:QS * (slab + 1)] = res.results[core]["out"]
    return out.reshape(B, C, 64, 64)


if __name__ == "__main__":
    rng = np.random.default_rng(0)
    inputs = {
        "x": rng.standard_normal((2, C, 64, 64)).astype(np.float32),
        "gn_w": np.ones(C, np.float32),
        "gn_b": np.zeros(C, np.float32),
    }
    for nm in ("q", "k", "v", "p"):
        inputs[f"w{nm}"] = (rng.standard_normal((C, C)) * 0.02).astype(np.float32)
        inputs[f"b{nm}"] = np.zeros(C, np.float32)
    out = kernel(**inputs)
    print("ran:", out.shape, out.dtype)


# revision 12
# speedup vs baseline: 2.2625x; 1.1005x over previous
"""AttnBlock (GroupNorm -> QKV -> 4096x4096 single-head attention -> proj ->
residual) on 8 TRN2 NeuronCores.

Sharding: data-parallel over batch (B=2) x sequence-parallel over query
positions (4 slabs of 1024). Each core receives the full x[b] (rolled so its
query slab sits at columns 0:1024), computes GroupNorm + v for the whole
image (replicated within the 4-core batch group -> zero collectives), and
attention + projection + residual for its 1024 query columns only.

Precision/structure highlights:
- All heavy matmuls are fp8e4m3 DoubleRow (2x128 contraction / instr at
  0.5 cyc/row), fp32 PSUM.
- k is never materialized: score = (Wk^T q)^T . h, so we compute
  u = Wk^T q_f8 (C x 1024) and use the fp8 x itself as the score
  stationary. The bk term is per-query constant -> softmax invariant ->
  dropped.
- GroupNorm folds: scale -> folded into wq/wv weight tiles on device
  (per-partition) and into the u drain; shift -> tiny N=1 bias-chain
  matmuls (bq2 = 32(Wq shift + bq)) plus a per-channel residual constant
  out += Wp (Wv shift + bv) handled via x_sl. GN stats use a stride-8
  subsample (var estimator err ~1.6%, far below fp8 noise).
- Softmax denominators via a DoubleRow matmul with constant-4.0
  stationary; the o drain fuses the normalization (tensor_tensor with the
  broadcast reciprocal).
"""
import sys
sys.path.insert(0, '/opt/trn_rl_repo')
import contextlib
import numpy as np
import ml_dtypes

import concourse.bass as bass
import concourse.tile as tile
from concourse import mybir, bacc
from concourse import bass_utils

f32 = mybir.dt.float32
bf16 = mybir.dt.bfloat16
fp8 = mybir.dt.float8e4
AF = mybir.ActivationFunctionType
ALU = mybir.AluOpType
DR = mybir.MatmulPerfMode.DoubleRow
F8 = ml_dtypes.float8_e4m3

C = 512          # channels
N = 4096         # positions (64*64)
G = 32           # groupnorm groups
GP = 16          # channels per group
NT = C // 128    # 4 channel partition-tiles
QS = 1024        # query slab per core
QC = 256         # query chunk in attention phase
EPS = 1e-6
SW = 32.0        # weight scale folded into all four fp8 weights
SEXP = 1.0 / (16.0 * float(np.sqrt(C)))  # score_psum = 16 * qk/1 -> *SEXP
OSC = 2.0 ** -8  # final pp drain scale (pp = 256 * wp.o_norm)
STRIDE = 8       # GN stats subsample stride
NSAMP = float(GP * (N // STRIDE))        # samples per group


def _emit_body(nc, tc, p, x8_d, xsl_d, wall_d, misc_d, out):
    sb, scr, pq, outp, small = p["sb"], p["scr"], p["pq"], p["outp"], p["small"]
    ps_mm, ps_o, ps_sums, ps_sm = p["ps_mm"], p["ps_o"], p["ps_sums"], p["ps_sm"]

    ones_row = small.tile([1, 128], bf16, tag="ones_row")
    nc.vector.memset(ones_row[:], 1.0)
    ones4 = small.tile([128, 2, 32], fp8, tag="ones4")
    nc.vector.memset(ones4[:], 4.0)
    eps8 = small.tile([8, 1], f32, tag="eps8")
    nc.vector.memset(eps8[:], EPS)

    # ---- phase 0: DMA + GN stats ---------------------------------
    x8 = sb.tile([128, NT, N], fp8, tag="x8")
    nc.sync.dma_start(x8[:], x8_d.ap().rearrange("(t p) n -> p t n", p=128))
    wall = sb.tile([128, 4, NT, C], fp8, tag="wall")
    nc.sync.dma_start(wall[:], wall_d.ap())
    misc = sb.tile([128, 672], f32, tag="misc")
    nc.sync.dma_start(misc[:], misc_d.ap())

    w_q, w_kT, w_v, w_p = (wall[:, i, :, :] for i in range(4))
    bq32 = [misc[:, t:t + 1] for t in range(NT)]          # 32*bq
    bv32 = [misc[:, 4 + t:5 + t] for t in range(NT)]      # 32*bv (cout tiles)
    bp_t = [misc[:, 8 + t:9 + t] for t in range(NT)]
    gnw_t = [misc[:, 12 + t:13 + t] for t in range(NT)]
    gnb_t = [misc[:, 16 + t:17 + t] for t in range(NT)]
    sel8p = misc[:, 20:148]        # [128,128] group selector, zero-padded
    sel8T = misc[0:8, 160:288]

    # stats on stride-8 subsample: sum on DVE, sumsq on ACT
    stats2 = small.tile([128, 8], f32, tag="stats2")      # cols 0:4 sum, 4:8 sq
    for t in range(NT):
        samp = x8[:, t, :].rearrange("p (n s) -> p s n", s=STRIDE)[:, 0, :]
        nc.vector.reduce_sum(stats2[:, t:t + 1], samp, axis=mybir.AxisListType.X)
        sqs = scr.tile([128, N // STRIDE], bf16, tag="sqs")
        nc.scalar.activation(sqs[:], samp, AF.Square,
                             accum_out=stats2[:, 4 + t:5 + t])

    # batched GN finalize: one Ln + one Exp for all 32 groups
    ps_all = ps_sm.tile([128, 512], f32, tag="sm")    # one bank for all
    # sel8p is zero-padded to 128 stationary columns so this first matmul
    # covers all 128 partitions: its start=True pending-zeroes the whole
    # bank for every later start=False matmul below.
    ps_g = ps_all[:, 0:8]
    nc.tensor.matmul(ps_g, sel8p, stats2[:], start=True, stop=True)
    mst = small.tile([8, 8], f32, tag="mst")
    nc.scalar.mul(mst[:], ps_g[0:8, :], 1.0 / NSAMP)           # mean t | ex2 t
    var4 = small.tile([8, 4], f32, tag="var4")
    m2 = small.tile([8, 4], f32, tag="m2")
    nc.vector.tensor_tensor(m2[:], mst[:, 0:4], mst[:, 0:4], op=ALU.mult)
    nc.vector.tensor_tensor(var4[:], mst[:, 4:8], m2[:], op=ALU.subtract)
    lnv = small.tile([8, 4], f32, tag="lnv")
    nc.scalar.activation(lnv[:], var4[:], AF.Ln, bias=eps8[:])
    grp2 = small.tile([8, 2, NT], f32, tag="grp2")        # mean row | rstd row
    nc.vector.tensor_copy(grp2[:, 0, :], mst[:, 0:4])
    nc.scalar.activation(grp2[:, 1, :], lnv[:], AF.Exp, scale=-0.5)
    # NOTE: ps_g's start=True zeroed this whole PSUM bank; every later
    # matmul into the ps_sm bank must use start=False (pending-zero) so it
    # does not clobber earlier results that are still being read.
    ps_bc = ps_all[:, 8:16]                               # mean t | rstd t
    nc.tensor.matmul(ps_bc, sel8T, grp2[:].rearrange("p a b -> p (a b)"),
                     start=False, stop=True)
    # scale/shift per tile + derived columns
    scale_t, sc6_t = [], []
    sh8 = small.tile([128, NT], fp8, tag="sh8")           # 128*shift, fp8
    for t in range(NT):
        sc = small.tile([128, 1], f32, tag=f"scale{t}")
        nc.vector.tensor_tensor(sc[:], gnw_t[t], ps_bc[:, 4 + t:5 + t],
                                op=ALU.mult)
        sc6 = small.tile([128, 1], f32, tag=f"sc6{t}")
        nc.vector.tensor_scalar_mul(sc6[:], sc[:], 2.0 ** -6)
        nsc = small.tile([128, 1], f32, tag=f"nscale{t}")
        nc.vector.tensor_scalar_mul(nsc[:], sc[:], -1.0)
        sh = small.tile([128, 1], f32, tag=f"shift{t}")
        nc.vector.scalar_tensor_tensor(sh[:], ps_bc[:, t:t + 1], nsc[:],
                                       gnb_t[t], op0=ALU.mult, op1=ALU.add)
        nc.vector.tensor_scalar_mul(sh8[:, t:t + 1], sh[:], 128.0)
        scale_t.append(sc)
        sc6_t.append(sc6)

    # bias chains (N=1 fp8 matmuls, before the in-place weight folds):
    # bq2 = 32(Wq shift + bq);  vc = 2^5 (Wv shift + bv);
    # badd = Wp vc / 2^10  (-> residual constant Wp(Wv shift + bv))
    ps_bq = ps_all[:, 16:20]
    ps_vc = ps_all[:, 20:24]
    for t in range(NT):
        for s in range(NT):
            nc.tensor.matmul(ps_bq[:, t:t + 1],
                             w_q[:, s, t * 128:(t + 1) * 128], sh8[:, s:s + 1],
                             start=False, stop=(s == 3))
            nc.tensor.matmul(ps_vc[:, t:t + 1],
                             w_v[:, s, t * 128:(t + 1) * 128], sh8[:, s:s + 1],
                             start=False, stop=(s == 3))
    bq2 = small.tile([128, 4], f32, tag="bq2")
    for t in range(NT):
        nc.vector.tensor_scalar(bq2[:, t:t + 1], ps_bq[:, t:t + 1],
                                2.0 ** -7, bq32[t], op0=ALU.mult, op1=ALU.add)
    vc8 = small.tile([128, 4], fp8, tag="vc8")
    for t in range(NT):
        nc.scalar.activation(vc8[:, t:t + 1], ps_vc[:, t:t + 1], AF.Identity,
                             scale=2.0 ** -7, bias=bv32[t])
    ps_t2 = ps_all[:, 24:28]
    for t in range(NT):
        for s in range(NT):
            nc.tensor.matmul(ps_t2[:, t:t + 1],
                             w_p[:, s, t * 128:(t + 1) * 128], vc8[:, s:s + 1],
                             start=False, stop=(s == 3))
    badd = small.tile([128, 4], f32, tag="badd")
    for t in range(NT):
        nc.vector.tensor_scalar(badd[:, t:t + 1], ps_t2[:, t:t + 1],
                                2.0 ** -10, bp_t[t], op0=ALU.mult, op1=ALU.add)

    # fold GN scale into wq / wv (per-partition, in place, Pool)
    for s in range(NT):
        nc.gpsimd.tensor_scalar_mul(w_q[:, s, :], w_q[:, s, :], scale_t[s][:])
        nc.gpsimd.tensor_scalar_mul(w_v[:, s, :], w_v[:, s, :], scale_t[s][:])

    # residual slab + per-channel constant (bp + Wp(Wv shift + bv))
    x_sl = sb.tile([128, NT, QS], f32, tag="x_sl")
    nc.sync.dma_start(x_sl[:], xsl_d.ap().rearrange("(t p) n -> p t n", p=128))
    for t in range(NT):
        nc.gpsimd.tensor_scalar_add(x_sl[:, t, :], x_sl[:, t, :],
                                    badd[:, t:t + 1])

    # ---- phase 1: q, u = Wk^T q (drains alternate ACT/DVE) -------
    q_f8 = sb.tile([128, NT, QS], fp8, tag="q_f8")
    for t in range(NT):
        psq = ps_mm.tile([128, 1024], f32, tag="mm")
        for qc2 in range(4):
            for a in range(2):
                nc.tensor.matmul(psq[:, qc2 * 256:(qc2 + 1) * 256],
                                 w_q[:, 2 * a:2 * a + 2, t * 128:(t + 1) * 128],
                                 x8[:, 2 * a:2 * a + 2, qc2 * 256:(qc2 + 1) * 256],
                                 start=(a == 0 and qc2 % 2 == 0), stop=(a == 1),
                                 perf_mode=DR)
        if t % 2 == 0:
            nc.scalar.activation(q_f8[:, t, :], psq[:], AF.Identity,
                                 bias=bq2[:, t:t + 1])
        else:
            nc.vector.tensor_scalar_add(q_f8[:, t, :], psq[:], bq2[:, t:t + 1])

    u_f8 = sb.tile([128, NT, QS], fp8, tag="u_f8")
    for t in range(NT):
        psu = ps_mm.tile([128, 1024], f32, tag="mm")
        for qc2 in range(4):
            for a in range(2):
                nc.tensor.matmul(psu[:, qc2 * 256:(qc2 + 1) * 256],
                                 w_kT[:, 2 * a:2 * a + 2, t * 128:(t + 1) * 128],
                                 q_f8[:, 2 * a:2 * a + 2, qc2 * 256:(qc2 + 1) * 256],
                                 start=(a == 0 and qc2 % 2 == 0), stop=(a == 1),
                                 perf_mode=DR)
        if t % 2 == 0:
            nc.scalar.mul(u_f8[:, t, :], psu[:], sc6_t[t][:])
        else:
            nc.vector.tensor_scalar_mul(u_f8[:, t, :], psu[:], sc6_t[t][:])

    # ---- phase 2 ---------------------------------------------------
    # v-projection is interleaved into qch 0 (the vt tiles for quad qd are
    # drained two quads before their o-matmuls need them). Quads are
    # software-pipelined with a skew of 2 (o/sums of quad qd-2 are emitted
    # after scores+exp of quad qd) and each qch's scalar tail (reciprocal,
    # o drain, proj, residual) is deferred until two quads into the NEXT
    # qch, so PE/ACT never sit in the serial tail chain.
    vt_f8 = sb.tile([128, N // 128, C], fp8, tag="vt_f8")

    def emit_v(np_, eng):
        psv = ps_mm.tile([128, 2, C], f32, tag="mm")
        for j in range(2):
            nt = 2 * np_ + j
            for co in range(2):
                for a in range(2):
                    nc.tensor.matmul(psv[:, j, co * 256:(co + 1) * 256],
                                     x8[:, 2 * a:2 * a + 2, nt * 128:(nt + 1) * 128],
                                     w_v[:, 2 * a:2 * a + 2, co * 256:(co + 1) * 256],
                                     start=(co == 0 and a == 0), stop=(a == 1),
                                     perf_mode=DR)
        if eng == "dve":
            nc.vector.tensor_copy(vt_f8[:, 2 * np_:2 * np_ + 2, :], psv[:])
        else:
            nc.scalar.copy(vt_f8[:, 2 * np_:2 * np_ + 2, :], psv[:])

    def flush(st, pp_q, qd):
        ob, sums_ps = st
        for a2 in range(2):
            kt0 = qd * 4 + 2 * a2
            first = (qd == 0 and a2 == 0)
            last = (qd == 7 and a2 == 1)
            for ct in range(4):
                nc.tensor.matmul(ob[ct // 2][:, ct % 2, :],
                                 vt_f8[:, kt0:kt0 + 2, ct * 128:(ct + 1) * 128],
                                 pp_q[:, 2 * a2:2 * a2 + 2, :],
                                 start=(first and ct % 2 == 0), stop=last,
                                 perf_mode=DR)
            nc.tensor.matmul(sums_ps[:, 0:QC], ones4[:],
                             pp_q[:, 2 * a2:2 * a2 + 2, :],
                             start=first, stop=last, perf_mode=DR)

    def tail(qch, st):
        ob, sums_ps = st
        q0 = qch * QC
        r_sb = small.tile([1, QC], f32, name="r_sb", tag="r_sb")
        nc.vector.reciprocal(r_sb[:], sums_ps[0:1, 0:QC])
        r_bf = small.tile([1, QC], bf16, name="r_bf", tag="r_bf")
        nc.vector.tensor_copy(r_bf[:], r_sb[:])
        ps_r = ps_mm.tile([128, 1024], f32, name="ps_r", tag="mm")
        nc.tensor.matmul(ps_r[:, 0:QC], ones_row[:], r_bf[:],
                         start=True, stop=True)
        r_bc2 = small.tile([128, 2, QC], f32, name="r_bc2", tag="r_bc2")
        nc.vector.tensor_copy(r_bc2[:, 0, :], ps_r[:, 0:QC])
        nc.vector.tensor_copy(r_bc2[:, 1, :], ps_r[:, 0:QC])
        # o drain fuses softmax normalization: o_f8 = ob * r (= 8 * o_norm)
        o_f8 = scr.tile([128, 4, QC], fp8, name="o_f8", tag="o_f8")
        nc.vector.tensor_tensor(o_f8[:, 0:2, :], ob[0][:], r_bc2[:], op=ALU.mult)
        nc.vector.tensor_tensor(o_f8[:, 2:4, :], ob[1][:], r_bc2[:], op=ALU.mult)
        for half in range(2):
            ppb = ps_o.tile([128, 2, QC], f32, name=f"pp{half}", tag=f"ob{half}")
            for tl in range(2):
                t = 2 * half + tl
                for a in range(2):
                    nc.tensor.matmul(ppb[:, tl, :],
                                     w_p[:, 2 * a:2 * a + 2, t * 128:(t + 1) * 128],
                                     o_f8[:, 2 * a:2 * a + 2, :],
                                     start=(tl == 0 and a == 0), stop=(a == 1),
                                     perf_mode=DR)
            outb = outp.tile([128, 2, QC], f32, name="outb", tag="outb")
            nc.vector.scalar_tensor_tensor(
                outb[:], ppb[:], OSC,
                x_sl[:, 2 * half:2 * half + 2, q0:q0 + QC],
                op0=ALU.mult, op1=ALU.add)
            for tl in range(2):
                nc.sync.dma_start(
                    out.ap()[(2 * half + tl) * 128:(2 * half + tl + 1) * 128,
                             q0:q0 + QC],
                    outb[:, tl, :])

    prev_st = None
    for qch in range(QS // QC):
        q0 = qch * QC
        st = None
        pends = {}
        for qd in range(8):
            if qch == 0:
                emit_v(2 * qd, "dve")
                emit_v(2 * qd + 1, "act" if qd % 2 == 0 else "dve")
            stq = ps_mm.tile([128, 4, QC], f32, name="stq", tag="mm")
            for j in range(4):
                kt = qd * 4 + j
                for a in range(2):
                    nc.tensor.matmul(stq[:, j, :],
                                     x8[:, 2 * a:2 * a + 2, kt * 128:(kt + 1) * 128],
                                     u_f8[:, 2 * a:2 * a + 2, q0:q0 + QC],
                                     start=(j % 2 == 0 and a == 0), stop=(a == 1),
                                     perf_mode=DR)
            p_q = pq.tile([128, 4, QC], fp8, name="p_q", tag="p")
            nc.scalar.activation(p_q[:], stq[:], AF.Exp, scale=SEXP)
            pends[qd] = p_q
            if qd == 2 and prev_st is not None:
                tail(qch - 1, prev_st)
            if qd >= 2:
                if st is None:
                    st = ([ps_o.tile([128, 2, QC], f32, name=f"ob{h}",
                                     tag=f"ob{h}") for h in range(2)],
                          ps_sums.tile([32, 512], f32, name="sums", tag="sums"))
                flush(st, pends.pop(qd - 2), qd - 2)
        flush(st, pends.pop(6), 6)
        flush(st, pends.pop(7), 7)
        prev_st = st
    tail(3, prev_st)


def _make_pools(tc, ctx):
    p = {}
    p["sb"] = ctx.enter_context(tc.tile_pool(name="sb", bufs=1))
    p["scr"] = ctx.enter_context(tc.tile_pool(name="scr", bufs=2))
    p["pq"] = ctx.enter_context(tc.tile_pool(name="pq", bufs=3))
    p["outp"] = ctx.enter_context(tc.tile_pool(name="outp", bufs=2))
    p["small"] = ctx.enter_context(tc.tile_pool(name="small", bufs=1))
    p["ps_mm"] = ctx.enter_context(tc.tile_pool(name="ps_mm", bufs=2, space="PSUM"))
    p["ps_o"] = ctx.enter_context(tc.tile_pool(name="ps_o", bufs=1, space="PSUM"))
    p["ps_sums"] = ctx.enter_context(tc.tile_pool(name="ps_sums", bufs=1, space="PSUM"))
    p["ps_sm"] = ctx.enter_context(tc.tile_pool(name="ps_sm", bufs=1, space="PSUM"))
    return p


def _build():
    nc = bacc.Bacc("TRN2", target_bir_lowering=False, debug=False, num_devices=8)
    x8_d = nc.dram_tensor("x8", [C, N], fp8, kind="ExternalInput")
    xsl_d = nc.dram_tensor("xsl", [C, QS], f32, kind="ExternalInput")
    wall_d = nc.dram_tensor("wall", [128, 4 * NT * C], fp8, kind="ExternalInput")
    misc_d = nc.dram_tensor("misc", [128, 672], f32, kind="ExternalInput")
    out = nc.dram_tensor("out", [C, QS], f32, kind="ExternalOutput")
    with tile.TileContext(nc) as tc:
        with contextlib.ExitStack() as ctx:
            p = _make_pools(tc, ctx)
            _emit_body(nc, tc, p, x8_d, xsl_d, wall_d, misc_d, out)
    nc.compile()
    return nc


_NC = None


def _get_nc():
    global _NC
    if _NC is None:
        _NC = _build()
    return _NC


def kernel(x, gn_w, gn_b, wq, bq, wk, bk, wv, bv, wp, bp):
    x = np.asarray(x, dtype=np.float32)
    B = x.shape[0]
    assert x.shape == (B, C, 64, 64)

    # weight slots: q/v/p transposed ([p, cin_sub, cout] = 32*W[cout, .]),
    # k slot NOT transposed ([p, cout_sub, cin] = 32*W[cout_sub*128+p, cin])
    # since it is the stationary of u = Wk^T q.
    wall = np.zeros((128, 4, NT, C), F8)
    for iw, (w, tr) in enumerate(((wq, True), (wk, False), (wv, True),
                                 (wp, True))):
        wm = np.asarray(w, np.float32) * SW
        if tr:
            wm = wm.T
        wall[:, iw, :, :] = wm.reshape(NT, 128, C).transpose(1, 0, 2).astype(F8)
    wall = wall.reshape(128, 4 * NT * C)

    misc = np.zeros((128, 672), np.float32)
    for t in range(NT):
        sl = slice(t * 128, (t + 1) * 128)
        misc[:, t] = np.asarray(bq, np.float32)[sl] * SW
        misc[:, 4 + t] = np.asarray(bv, np.float32)[sl] * SW
        misc[:, 8 + t] = np.asarray(bp, np.float32)[sl]
        misc[:, 12 + t] = np.asarray(gn_w, np.float32)[sl]
        misc[:, 16 + t] = np.asarray(gn_b, np.float32)[sl]
    sel8 = np.zeros((128, 8), np.float32)
    for pp_ in range(128):
        sel8[pp_, pp_ // GP] = 1.0
    misc[:, 20:148] = np.concatenate(
        [sel8, np.zeros((128, 120), np.float32)], axis=1)
    misc[0:8, 160:288] = sel8.T

    xf = x.reshape(B, C, N)
    in_maps = []
    for core in range(8):
        b, slab = core // 4, core % 4
        xr = np.roll(xf[b], -QS * slab, axis=1)
        in_maps.append({
            "x8": np.ascontiguousarray(xr).astype(F8),
            "xsl": np.ascontiguousarray(xr[:, 0:QS]),
            "wall": wall, "misc": misc,
        })

    nc = _get_nc()
    res = bass_utils.run_bass_kernel_spmd(nc, in_maps, core_ids=list(range(8)))

    out = np.empty((B, C, N), np.float32)
    for core in range(8):
        b, slab = core // 4, core % 4
        out[b][:, QS * slab:QS * (slab + 1)] = res.results[core]["out"]
    return out.reshape(B, C, 64, 64)


if __name__ == "__main__":
    rng = np.random.default_rng(0)
    inputs = {
        "x": rng.standard_normal((2, C, 64, 64)).astype(np.float32),
        "gn_w": np.ones(C, np.float32),
        "gn_b": np.zeros(C, np.float32),
    }
    for nm in ("q", "k", "v", "p"):
        inputs[f"w{nm}"] = (rng.standard_normal((C, C)) * 0.02).astype(np.float32)
        inputs[f"b{nm}"] = np.zeros(C, np.float32)
    out = kernel(**inputs)
    print("ran:", out.shape, out.dtype)


# revision 14
# speedup vs baseline: 2.3550x; 1.0409x over previous
"""AttnBlock (GroupNorm -> QKV -> 4096x4096 single-head attention -> proj ->
residual) on 8 TRN2 NeuronCores.

Sharding: data-parallel over batch (B=2) x sequence-parallel over query
positions (4 slabs of 1024). Each core receives the full x[b] (rolled so its
query slab sits at columns 0:1024), computes GroupNorm + v for the whole
image (replicated within the 4-core batch group -> zero collectives), and
attention + projection + residual for its 1024 query columns only.

Precision/structure highlights:
- All heavy matmuls are fp8e4m3 DoubleRow (2x128 contraction / instr at
  0.5 cyc/row), fp32 PSUM.
- k is never materialized: score = (Wk^T q)^T . h, so we compute
  u = Wk^T q_f8 (C x 1024) and use the fp8 x itself as the score
  stationary. The bk term is per-query constant -> softmax invariant ->
  dropped.
- GroupNorm folds: scale -> folded into wq/wv weight tiles on device
  (per-partition) and into the u drain; shift -> tiny N=1 bias-chain
  matmuls (bq2 = 32(Wq shift + bq)) plus a per-channel residual constant
  out += Wp (Wv shift + bv) handled via x_sl. GN stats use a stride-8
  subsample (var estimator err ~1.6%, far below fp8 noise).
- Softmax denominators via a DoubleRow matmul with constant-4.0
  stationary; the o drain fuses the normalization (tensor_tensor with the
  broadcast reciprocal).
"""
import sys
sys.path.insert(0, '/opt/trn_rl_repo')
import contextlib
import numpy as np
import ml_dtypes

import concourse.bass as bass
import concourse.tile as tile
from concourse import mybir, bacc
from concourse import bass_utils

f32 = mybir.dt.float32
bf16 = mybir.dt.bfloat16
fp8 = mybir.dt.float8e4
AF = mybir.ActivationFunctionType
ALU = mybir.AluOpType
DR = mybir.MatmulPerfMode.DoubleRow
F8 = ml_dtypes.float8_e4m3

C = 512          # channels
N = 4096         # positions (64*64)
G = 32           # groupnorm groups
GP = 16          # channels per group
NT = C // 128    # 4 channel partition-tiles
QS = 1024        # query slab per core
QC = 256         # query chunk in attention phase
EPS = 1e-6
SW = 32.0        # weight scale folded into all four fp8 weights
SEXP = 1.0 / (16.0 * float(np.sqrt(C)))  # score_psum = 16 * qk/1 -> *SEXP
OSC = 2.0 ** -8  # final pp drain scale (pp = 256 * wp.o_norm)
STRIDE = 8       # GN stats subsample stride
NSAMP = float(GP * (N // STRIDE))        # samples per group


def _emit_body(nc, tc, p, x8_d, xsl_d, wall_d, misc_d, out):
    sb, scr, pq, outp, small = p["sb"], p["scr"], p["pq"], p["outp"], p["small"]
    ps_mm, ps_o, ps_sums, ps_sm = p["ps_mm"], p["ps_o"], p["ps_sums"], p["ps_sm"]

    ones_row = small.tile([1, 128], bf16, tag="ones_row")
    nc.vector.memset(ones_row[:], 1.0)
    ones4 = small.tile([128, 2, 32], fp8, tag="ones4")
    nc.vector.memset(ones4[:], 4.0)
    eps8 = small.tile([8, 1], f32, tag="eps8")
    nc.vector.memset(eps8[:], EPS)

    # ---- phase 0: DMA + GN stats ---------------------------------
    xr = x8_d.ap().rearrange("(t p) n -> p t n", p=128)
    x8a = sb.tile([128, 2, N], fp8, tag="x8a")
    nc.sync.dma_start(x8a[:], xr[:, 0:2, :])
    wall = sb.tile([128, 4, NT, C], fp8, tag="wall")
    nc.sync.dma_start(wall[:], wall_d.ap())
    misc = sb.tile([128, 672], f32, tag="misc")
    nc.sync.dma_start(misc[:], misc_d.ap())
    x8b = sb.tile([128, 2, N], fp8, tag="x8b")
    nc.sync.dma_start(x8b[:], xr[:, 2:4, :])
    xpair = [x8a, x8b]

    w_q, w_kT, w_v, w_p = (wall[:, i, :, :] for i in range(4))
    bq32 = [misc[:, t:t + 1] for t in range(NT)]          # 32*bq
    bv32 = [misc[:, 4 + t:5 + t] for t in range(NT)]      # 32*bv (cout tiles)
    bp_t = [misc[:, 8 + t:9 + t] for t in range(NT)]
    gnw_t = [misc[:, 12 + t:13 + t] for t in range(NT)]
    gnb_t = [misc[:, 16 + t:17 + t] for t in range(NT)]
    sel8p = misc[:, 20:148]        # [128,128] group selector, zero-padded
    sel8T = misc[0:8, 160:288]

    # stats on stride-8 subsample: sum on DVE, sumsq on ACT
    stats2 = small.tile([128, 8], f32, tag="stats2")      # cols 0:4 sum, 4:8 sq
    for t in range(NT):
        samp = xpair[t // 2][:, t % 2, :].rearrange(
            "p (n s) -> p s n", s=STRIDE)[:, 0, :]
        nc.vector.reduce_sum(stats2[:, t:t + 1], samp, axis=mybir.AxisListType.X)
        sqs = scr.tile([128, N // STRIDE], bf16, tag="sqs")
        nc.scalar.activation(sqs[:], samp, AF.Square,
                             accum_out=stats2[:, 4 + t:5 + t])

    # batched GN finalize: one Ln + one Exp for all 32 groups
    ps_all = ps_sm.tile([128, 512], f32, tag="sm")    # one bank for all
    # sel8p is zero-padded to 128 stationary columns so this first matmul
    # covers all 128 partitions: its start=True pending-zeroes the whole
    # bank for every later start=False matmul below.
    ps_g = ps_all[:, 0:8]
    nc.tensor.matmul(ps_g, sel8p, stats2[:], start=True, stop=True)
    mst = small.tile([8, 8], f32, tag="mst")
    nc.scalar.mul(mst[:], ps_g[0:8, :], 1.0 / NSAMP)           # mean t | ex2 t
    var4 = small.tile([8, 4], f32, tag="var4")
    m2 = small.tile([8, 4], f32, tag="m2")
    nc.vector.tensor_tensor(m2[:], mst[:, 0:4], mst[:, 0:4], op=ALU.mult)
    nc.vector.tensor_tensor(var4[:], mst[:, 4:8], m2[:], op=ALU.subtract)
    lnv = small.tile([8, 4], f32, tag="lnv")
    nc.scalar.activation(lnv[:], var4[:], AF.Ln, bias=eps8[:])
    grp2 = small.tile([8, 2, NT], f32, tag="grp2")        # mean row | rstd row
    nc.vector.tensor_copy(grp2[:, 0, :], mst[:, 0:4])
    nc.scalar.activation(grp2[:, 1, :], lnv[:], AF.Exp, scale=-0.5)
    # NOTE: ps_g's start=True zeroed this whole PSUM bank; every later
    # matmul into the ps_sm bank must use start=False (pending-zero) so it
    # does not clobber earlier results that are still being read.
    ps_bc = ps_all[:, 8:16]                               # mean t | rstd t
    nc.tensor.matmul(ps_bc, sel8T, grp2[:].rearrange("p a b -> p (a b)"),
                     start=False, stop=True)
    # scale/shift per tile + derived columns
    scale_t, sc6_t = [], []
    sh8 = small.tile([128, NT], fp8, tag="sh8")           # 128*shift, fp8
    for t in range(NT):
        sc = small.tile([128, 1], f32, tag=f"scale{t}")
        nc.vector.tensor_tensor(sc[:], gnw_t[t], ps_bc[:, 4 + t:5 + t],
                                op=ALU.mult)
        sc6 = small.tile([128, 1], f32, tag=f"sc6{t}")
        nc.vector.tensor_scalar_mul(sc6[:], sc[:], 2.0 ** -6)
        nsc = small.tile([128, 1], f32, tag=f"nscale{t}")
        nc.vector.tensor_scalar_mul(nsc[:], sc[:], -1.0)
        sh = small.tile([128, 1], f32, tag=f"shift{t}")
        nc.vector.scalar_tensor_tensor(sh[:], ps_bc[:, t:t + 1], nsc[:],
                                       gnb_t[t], op0=ALU.mult, op1=ALU.add)
        nc.vector.tensor_scalar_mul(sh8[:, t:t + 1], sh[:], 128.0)
        scale_t.append(sc)
        sc6_t.append(sc6)

    # bias chains (N=1 fp8 matmuls, before the in-place weight folds):
    # bq2 = 32(Wq shift + bq);  vc = 2^5 (Wv shift + bv);
    # badd = Wp vc / 2^10  (-> residual constant Wp(Wv shift + bv))
    ps_bq = ps_all[:, 16:20]
    ps_vc = ps_all[:, 20:24]
    for t in range(NT):
        for s in range(NT):
            nc.tensor.matmul(ps_bq[:, t:t + 1],
                             w_q[:, s, t * 128:(t + 1) * 128], sh8[:, s:s + 1],
                             start=False, stop=(s == 3))
            nc.tensor.matmul(ps_vc[:, t:t + 1],
                             w_v[:, s, t * 128:(t + 1) * 128], sh8[:, s:s + 1],
                             start=False, stop=(s == 3))
    bq2 = small.tile([128, 4], f32, tag="bq2")
    for t in range(NT):
        nc.vector.tensor_scalar(bq2[:, t:t + 1], ps_bq[:, t:t + 1],
                                2.0 ** -7, bq32[t], op0=ALU.mult, op1=ALU.add)
    vc8 = small.tile([128, 4], fp8, tag="vc8")
    for t in range(NT):
        nc.scalar.activation(vc8[:, t:t + 1], ps_vc[:, t:t + 1], AF.Identity,
                             scale=2.0 ** -7, bias=bv32[t])
    ps_t2 = ps_all[:, 24:28]
    for t in range(NT):
        for s in range(NT):
            nc.tensor.matmul(ps_t2[:, t:t + 1],
                             w_p[:, s, t * 128:(t + 1) * 128], vc8[:, s:s + 1],
                             start=False, stop=(s == 3))
    badd = small.tile([128, 4], f32, tag="badd")
    for t in range(NT):
        nc.vector.tensor_scalar(badd[:, t:t + 1], ps_t2[:, t:t + 1],
                                2.0 ** -10, bp_t[t], op0=ALU.mult, op1=ALU.add)

    # fold GN scale into wq / wv (per-partition, in place, Pool)
    for s in range(NT):
        nc.gpsimd.tensor_scalar_mul(w_q[:, s, :], w_q[:, s, :], scale_t[s][:])
        nc.gpsimd.tensor_scalar_mul(w_v[:, s, :], w_v[:, s, :], scale_t[s][:])

    # residual slab + per-channel constant (bp + Wp(Wv shift + bv))
    x_sl = sb.tile([128, NT, QS], f32, tag="x_sl")
    nc.sync.dma_start(x_sl[:], xsl_d.ap().rearrange("(t p) n -> p t n", p=128))
    for t in range(NT):
        nc.gpsimd.tensor_scalar_add(x_sl[:, t, :], x_sl[:, t, :],
                                    badd[:, t:t + 1])

    # ---- phase 1: q, u = Wk^T q (drains alternate ACT/DVE) -------
    q_f8 = sb.tile([128, NT, QS], fp8, tag="q_f8")
    for t in range(NT):
        psq = ps_mm.tile([128, 1024], f32, tag="mm")
        for qc2 in range(4):
            for a in range(2):
                nc.tensor.matmul(psq[:, qc2 * 256:(qc2 + 1) * 256],
                                 w_q[:, 2 * a:2 * a + 2, t * 128:(t + 1) * 128],
                                 xpair[a][:, :, qc2 * 256:(qc2 + 1) * 256],
                                 start=(a == 0 and qc2 % 2 == 0), stop=(a == 1),
                                 perf_mode=DR)
        if t % 2 == 0:
            nc.scalar.activation(q_f8[:, t, :], psq[:], AF.Identity,
                                 bias=bq2[:, t:t + 1])
        else:
            nc.vector.tensor_scalar_add(q_f8[:, t, :], psq[:], bq2[:, t:t + 1])

    u_f8 = sb.tile([128, NT, QS], fp8, tag="u_f8")
    for t in range(NT):
        psu = ps_mm.tile([128, 1024], f32, tag="mm")
        for qc2 in range(4):
            for a in range(2):
                nc.tensor.matmul(psu[:, qc2 * 256:(qc2 + 1) * 256],
                                 w_kT[:, 2 * a:2 * a + 2, t * 128:(t + 1) * 128],
                                 q_f8[:, 2 * a:2 * a + 2, qc2 * 256:(qc2 + 1) * 256],
                                 start=(a == 0 and qc2 % 2 == 0), stop=(a == 1),
                                 perf_mode=DR)
        if t % 2 == 0:
            nc.scalar.mul(u_f8[:, t, :], psu[:], sc6_t[t][:])
        else:
            nc.vector.tensor_scalar_mul(u_f8[:, t, :], psu[:], sc6_t[t][:])

    # ---- phase 2 ---------------------------------------------------
    # v-projection is interleaved into qch 0 (the vt tiles for quad qd are
    # drained two quads before their o-matmuls need them). Quads are
    # software-pipelined with a skew of 2 (o/sums of quad qd-2 are emitted
    # after scores+exp of quad qd) and each qch's scalar tail (reciprocal,
    # o drain, proj, residual) is deferred until two quads into the NEXT
    # qch, so PE/ACT never sit in the serial tail chain.
    vt_f8 = sb.tile([128, N // 128, C], fp8, tag="vt_f8")

    def emit_v(np_, eng):
        psv = ps_mm.tile([128, 2, C], f32, tag="mm")
        for j in range(2):
            nt = 2 * np_ + j
            for co in range(2):
                for a in range(2):
                    nc.tensor.matmul(psv[:, j, co * 256:(co + 1) * 256],
                                     xpair[a][:, :, nt * 128:(nt + 1) * 128],
                                     w_v[:, 2 * a:2 * a + 2, co * 256:(co + 1) * 256],
                                     start=(co == 0 and a == 0), stop=(a == 1),
                                     perf_mode=DR)
        if eng == "dve":
            nc.vector.tensor_copy(vt_f8[:, 2 * np_:2 * np_ + 2, :], psv[:])
        else:
            nc.scalar.copy(vt_f8[:, 2 * np_:2 * np_ + 2, :], psv[:])

    def flush(st, pp_q, qd):
        ob, sums_ps = st
        for a2 in range(2):
            kt0 = qd * 4 + 2 * a2
            first = (qd == 0 and a2 == 0)
            last = (qd == 7 and a2 == 1)
            for ct in range(4):
                nc.tensor.matmul(ob[ct // 2][:, ct % 2, :],
                                 vt_f8[:, kt0:kt0 + 2, ct * 128:(ct + 1) * 128],
                                 pp_q[:, 2 * a2:2 * a2 + 2, :],
                                 start=(first and ct % 2 == 0), stop=last,
                                 perf_mode=DR)
            nc.tensor.matmul(sums_ps[:, 0:QC], ones4[:],
                             pp_q[:, 2 * a2:2 * a2 + 2, :],
                             start=first, stop=last, perf_mode=DR)

    def tail_a(st):
        ob, sums_ps = st
        r_sb = small.tile([1, QC], f32, name="r_sb", tag="r_sb")
        nc.vector.reciprocal(r_sb[:], sums_ps[0:1, 0:QC])
        r_bf = small.tile([1, QC], bf16, name="r_bf", tag="r_bf")
        nc.vector.tensor_copy(r_bf[:], r_sb[:])
        ps_r = ps_mm.tile([128, 1024], f32, name="ps_r", tag="mm")
        nc.tensor.matmul(ps_r[:, 0:QC], ones_row[:], r_bf[:],
                         start=True, stop=True)
        r_bc2 = small.tile([128, 2, QC], f32, name="r_bc2", tag="r_bc2")
        nc.vector.tensor_copy(r_bc2[:, 0, :], ps_r[:, 0:QC])
        nc.vector.tensor_copy(r_bc2[:, 1, :], ps_r[:, 0:QC])
        return r_bc2

    def tail_b(qch, st, r_bc2):
        ob, sums_ps = st
        q0 = qch * QC
        # o drain fuses softmax normalization: o_f8 = ob * r (= 8 * o_norm)
        o_f8 = scr.tile([128, 4, QC], fp8, name="o_f8", tag="o_f8")
        nc.vector.tensor_tensor(o_f8[:, 0:2, :], ob[0][:], r_bc2[:], op=ALU.mult)
        nc.vector.tensor_tensor(o_f8[:, 2:4, :], ob[1][:], r_bc2[:], op=ALU.mult)
        for half in range(2):
            ppb = ps_o.tile([128, 2, QC], f32, name=f"pp{half}", tag=f"ob{half}")
            for tl in range(2):
                t = 2 * half + tl
                for a in range(2):
                    nc.tensor.matmul(ppb[:, tl, :],
                                     w_p[:, 2 * a:2 * a + 2, t * 128:(t + 1) * 128],
                                     o_f8[:, 2 * a:2 * a + 2, :],
                                     start=(tl == 0 and a == 0), stop=(a == 1),
                                     perf_mode=DR)
            outb = outp.tile([128, 2, QC], f32, name="outb", tag="outb")
            nc.vector.scalar_tensor_tensor(
                outb[:], ppb[:], OSC,
                x_sl[:, 2 * half:2 * half + 2, q0:q0 + QC],
                op0=ALU.mult, op1=ALU.add)
            for tl in range(2):
                nc.sync.dma_start(
                    out.ap()[(2 * half + tl) * 128:(2 * half + tl + 1) * 128,
                             q0:q0 + QC],
                    outb[:, tl, :])

    prev_st = None
    prev_rbc = None
    for qch in range(QS // QC):
        q0 = qch * QC
        st = None
        pends = {}
        for qd in range(8):
            if qch == 0:
                emit_v(2 * qd, "dve")
                emit_v(2 * qd + 1, "act" if qd % 2 == 0 else "dve")
            stq = ps_mm.tile([128, 4, QC], f32, name="stq", tag="mm")
            for j in range(4):
                kt = qd * 4 + j
                for a in range(2):
                    nc.tensor.matmul(stq[:, j, :],
                                     xpair[a][:, :, kt * 128:(kt + 1) * 128],
                                     u_f8[:, 2 * a:2 * a + 2, q0:q0 + QC],
                                     start=(j % 2 == 0 and a == 0), stop=(a == 1),
                                     perf_mode=DR)
            p_q = pq.tile([128, 4, QC], fp8, name="p_q", tag="p")
            nc.scalar.activation(p_q[:], stq[:], AF.Exp, scale=SEXP)
            pends[qd] = p_q
            if qd == 0 and prev_st is not None:
                prev_rbc = tail_a(prev_st)
            if qd == 3 and prev_st is not None:
                tail_b(qch - 1, prev_st, prev_rbc)
            if qd >= 3:
                if st is None:
                    st = ([ps_o.tile([128, 2, QC], f32, name=f"ob{h}",
                                     tag=f"ob{h}") for h in range(2)],
                          ps_sums.tile([32, 512], f32, name="sums", tag="sums"))
                flush(st, pends.pop(qd - 3), qd - 3)
        for qd in (5, 6, 7):
            flush(st, pends.pop(qd), qd)
        prev_st = st
    prev_rbc = tail_a(prev_st)
    tail_b(3, prev_st, prev_rbc)


def _make_pools(tc, ctx):
    p = {}
    p["sb"] = ctx.enter_context(tc.tile_pool(name="sb", bufs=1))
    p["scr"] = ctx.enter_context(tc.tile_pool(name="scr", bufs=2))
    p["pq"] = ctx.enter_context(tc.tile_pool(name="pq", bufs=4))
    p["outp"] = ctx.enter_context(tc.tile_pool(name="outp", bufs=2))
    p["small"] = ctx.enter_context(tc.tile_pool(name="small", bufs=1))
    p["ps_mm"] = ctx.enter_context(tc.tile_pool(name="ps_mm", bufs=2, space="PSUM"))
    p["ps_o"] = ctx.enter_context(tc.tile_pool(name="ps_o", bufs=1, space="PSUM"))
    p["ps_sums"] = ctx.enter_context(tc.tile_pool(name="ps_sums", bufs=1, space="PSUM"))
    p["ps_sm"] = ctx.enter_context(tc.tile_pool(name="ps_sm", bufs=1, space="PSUM"))
    return p


def _build():
    nc = bacc.Bacc("TRN2", target_bir_lowering=False, debug=False, num_devices=8)
    x8_d = nc.dram_tensor("x8", [C, N], fp8, kind="ExternalInput")
    xsl_d = nc.dram_tensor("xsl", [C, QS], f32, kind="ExternalInput")
    wall_d = nc.dram_tensor("wall", [128, 4 * NT * C], fp8, kind="ExternalInput")
    misc_d = nc.dram_tensor("misc", [128, 672], f32, kind="ExternalInput")
    out = nc.dram_tensor("out", [C, QS], f32, kind="ExternalOutput")
    with tile.TileContext(nc) as tc:
        with contextlib.ExitStack() as ctx:
            p = _make_pools(tc, ctx)
            _emit_body(nc, tc, p, x8_d, xsl_d, wall_d, misc_d, out)
    nc.compile()
    return nc


_NC = None


def _get_nc():
    global _NC
    if _NC is None:
        _NC = _build()
    return _NC


def kernel(x, gn_w, gn_b, wq, bq, wk, bk, wv, bv, wp, bp):
    x = np.asarray(x, dtype=np.float32)
    B = x.shape[0]
    assert x.shape == (B, C, 64, 64)

    # weight slots: q/v/p transposed ([p, cin_sub, cout] = 32*W[cout, .]),
    # k slot NOT transposed ([p, cout_sub, cin] = 32*W[cout_sub*128+p, cin])
    # since it is the stationary of u = Wk^T q.
    wall = np.zeros((128, 4, NT, C), F8)
    for iw, (w, tr) in enumerate(((wq, True), (wk, False), (wv, True),
                                 (wp, True))):
        wm = np.asarray(w, np.float32) * SW
        if tr:
            wm = wm.T
        wall[:, iw, :, :] = wm.reshape(NT, 128, C).transpose(1, 0, 2).astype(F8)
    wall = wall.reshape(128, 4 * NT * C)

    misc = np.zeros((128, 672), np.float32)
    for t in range(NT):
        sl = slice(t * 128, (t + 1) * 128)
        misc[:, t] = np.asarray(bq, np.float32)[sl] * SW
        misc[:, 4 + t] = np.asarray(bv, np.float32)[sl] * SW
        misc[:, 8 + t] = np.asarray(bp, np.float32)[sl]
        misc[:, 12 + t] = np.asarray(gn_w, np.float32)[sl]
        misc[:, 16 + t] = np.asarray(gn_b, np.float32)[sl]
    sel8 = np.zeros((128, 8), np.float32)
    for pp_ in range(128):
        sel8[pp_, pp_ // GP] = 1.0
    misc[:, 20:148] = np.concatenate(
        [sel8, np.zeros((128, 120), np.float32)], axis=1)
    misc[0:8, 160:288] = sel8.T

    xf = x.reshape(B, C, N)
    in_maps = []
    for core in range(8):
        b, slab = core // 4, core % 4
        xr = np.roll(xf[b], -QS * slab, axis=1)
        in_maps.append({
            "x8": np.ascontiguousarray(xr).astype(F8),
            "xsl": np.ascontiguousarray(xr[:, 0:QS]),
            "wall": wall, "misc": misc,
        })

    nc = _get_nc()
    res = bass_utils.run_bass_kernel_spmd(nc, in_maps, core_ids=list(range(8)))

    out = np.empty((B, C, N), np.float32)
    for core in range(8):
        b, slab = core // 4, core % 4
        out[b][:, QS * slab:QS * (slab + 1)] = res.results[core]["out"]
    return out.reshape(B, C, 64, 64)


if __name__ == "__main__":
    rng = np.random.default_rng(0)
    inputs = {
        "x": rng.standard_normal((2, C, 64, 64)).astype(np.float32),
        "gn_w": np.ones(C, np.float32),
        "gn_b": np.zeros(C, np.float32),
    }
    for nm in ("q", "k", "v", "p"):
        inputs[f"w{nm}"] = (rng.standard_normal((C, C)) * 0.02).astype(np.float32)
        inputs[f"b{nm}"] = np.zeros(C, np.float32)
    out = kernel(**inputs)
    print("ran:", out.shape, out.dtype)


# revision 17
# speedup vs baseline: 2.5466x; 1.0814x over previous
"""AttnBlock (GroupNorm -> QKV -> 4096x4096 single-head attention -> proj ->
residual) on 8 TRN2 NeuronCores.

Sharding: data-parallel over batch (B=2) x sequence-parallel over query
positions (4 slabs of 1024). Each core receives the full x[b] (rolled so its
query slab sits at columns 0:1024), computes GroupNorm + v for the whole
image (replicated within the 4-core batch group -> zero collectives), and
attention + projection + residual for its 1024 query columns only.

Precision/structure highlights:
- All heavy matmuls are fp8e4m3 DoubleRow (2x128 contraction / instr at
  0.5 cyc/row), fp32 PSUM.
- k is never materialized: score = (Wk^T q)^T . h, so we compute
  u = Wk^T q_f8 (C x 1024) and use the fp8 x itself as the score
  stationary. The bk term is per-query constant -> softmax invariant ->
  dropped.
- GroupNorm folds: scale -> folded into wq/wv weight tiles on device
  (per-partition) and into the u drain; shift -> tiny N=1 bias-chain
  matmuls (bq2 = 32(Wq shift + bq)) plus a per-channel residual constant
  out += Wp (Wv shift + bv) handled via x_sl. GN stats use a stride-8
  subsample (var estimator err ~1.6%, far below fp8 noise).
- Softmax denominators via a DoubleRow matmul with constant-4.0
  stationary; the o drain fuses the normalization (tensor_tensor with the
  broadcast reciprocal).
"""
import sys
sys.path.insert(0, '/opt/trn_rl_repo')
import contextlib
import numpy as np
import ml_dtypes

import concourse.bass as bass
import concourse.tile as tile
from concourse import mybir, bacc
from concourse import bass_utils

f32 = mybir.dt.float32
bf16 = mybir.dt.bfloat16
fp8 = mybir.dt.float8e4
AF = mybir.ActivationFunctionType
ALU = mybir.AluOpType
DR = mybir.MatmulPerfMode.DoubleRow
F8 = ml_dtypes.float8_e4m3

C = 512          # channels
N = 4096         # positions (64*64)
G = 32           # groupnorm groups
GP = 16          # channels per group
NT = C // 128    # 4 channel partition-tiles
QS = 1024        # query slab per core
QC = 256         # query chunk in attention phase
EPS = 1e-6
SW = 32.0        # weight scale folded into all four fp8 weights
SEXP_LEGACY = 1.0 / (16.0 * float(np.sqrt(C)))
SEXP_FAST = 1.0 / (32.0 * float(np.sqrt(C)))
OSC = 2.0 ** -8  # final pp drain scale (pp = 256 * wp.o_norm)
STRIDE = 8       # GN stats subsample stride
NSAMP = float(GP * (N // STRIDE))        # samples per group


def _emit_body(nc, tc, p, x8_d, xsl_d, wall_d, misc_d, out, legacy_q):
    sb, scr, pq, outp, small = p["sb"], p["scr"], p["pq"], p["outp"], p["small"]
    ps_mm, ps_o, ps_sums, ps_sm = p["ps_mm"], p["ps_o"], p["ps_sums"], p["ps_sm"]

    ones_row = small.tile([1, 128], bf16, tag="ones_row")
    nc.vector.memset(ones_row[:], 1.0)
    ones4 = small.tile([128, 2, 32], fp8, tag="ones4")
    nc.vector.memset(ones4[:], 4.0)
    eps8 = small.tile([8, 1], f32, tag="eps8")
    nc.vector.memset(eps8[:], EPS)

    # ---- phase 0: DMA + GN stats ---------------------------------
    xr = x8_d.ap().rearrange("(t p) n -> p t n", p=128)
    x8a = sb.tile([128, 2, N], fp8, tag="x8a")
    nc.sync.dma_start(x8a[:], xr[:, 0:2, :])
    wall = sb.tile([128, 4, NT, C], fp8, tag="wall")
    nc.sync.dma_start(wall[:], wall_d.ap())
    misc = sb.tile([128, 672], f32, tag="misc")
    nc.sync.dma_start(misc[:], misc_d.ap())
    x8b = sb.tile([128, 2, N], fp8, tag="x8b")
    nc.sync.dma_start(x8b[:], xr[:, 2:4, :])
    xpair = [x8a, x8b]

    w_q, w_kT, w_v, w_p = (wall[:, i, :, :] for i in range(4))
    bq32 = [misc[:, t:t + 1] for t in range(NT)]          # 32*bq
    bv32 = [misc[:, 4 + t:5 + t] for t in range(NT)]      # 32*bv (cout tiles)
    bp_t = [misc[:, 8 + t:9 + t] for t in range(NT)]
    gnw_t = [misc[:, 12 + t:13 + t] for t in range(NT)]
    gnb_t = [misc[:, 16 + t:17 + t] for t in range(NT)]
    sel8p = misc[:, 20:148]        # [128,128] group selector, zero-padded
    sel8T = misc[0:8, 160:288]

    # stats on stride-8 subsample: sum on DVE, sumsq on ACT
    stats2 = small.tile([128, 8], f32, tag="stats2")      # cols 0:4 sum, 4:8 sq
    for t in range(NT):
        samp = xpair[t // 2][:, t % 2, :].rearrange(
            "p (n s) -> p s n", s=STRIDE)[:, 0, :]
        nc.vector.reduce_sum(stats2[:, t:t + 1], samp, axis=mybir.AxisListType.X)
        sqs = scr.tile([128, N // STRIDE], bf16, tag="sqs")
        nc.scalar.activation(sqs[:], samp, AF.Square,
                             accum_out=stats2[:, 4 + t:5 + t])

    # batched GN finalize: one Ln + one Exp for all 32 groups
    ps_all = ps_sm.tile([128, 512], f32, tag="sm")    # one bank for all
    # sel8p is zero-padded to 128 stationary columns so this first matmul
    # covers all 128 partitions: its start=True pending-zeroes the whole
    # bank for every later start=False matmul below.
    ps_g = ps_all[:, 0:8]
    nc.tensor.matmul(ps_g, sel8p, stats2[:], start=True, stop=True)
    mst = small.tile([8, 8], f32, tag="mst")
    nc.scalar.mul(mst[:], ps_g[0:8, :], 1.0 / NSAMP)           # mean t | ex2 t
    var4 = small.tile([8, 4], f32, tag="var4")
    m2 = small.tile([8, 4], f32, tag="m2")
    nc.vector.tensor_tensor(m2[:], mst[:, 0:4], mst[:, 0:4], op=ALU.mult)
    nc.vector.tensor_tensor(var4[:], mst[:, 4:8], m2[:], op=ALU.subtract)
    lnv = small.tile([8, 4], f32, tag="lnv")
    nc.scalar.activation(lnv[:], var4[:], AF.Ln, bias=eps8[:])
    grp2 = small.tile([8, 2, NT], f32, tag="grp2")        # mean row | rstd row
    nc.vector.tensor_copy(grp2[:, 0, :], mst[:, 0:4])
    nc.scalar.activation(grp2[:, 1, :], lnv[:], AF.Exp, scale=-0.5)
    # NOTE: ps_g's start=True zeroed this whole PSUM bank; every later
    # matmul into the ps_sm bank must use start=False (pending-zero) so it
    # does not clobber earlier results that are still being read.
    ps_bc = ps_all[:, 8:16]                               # mean t | rstd t
    nc.tensor.matmul(ps_bc, sel8T, grp2[:].rearrange("p a b -> p (a b)"),
                     start=False, stop=True)
    # scale/shift per tile + derived columns
    scale_t, sc6_t = [], []
    sh8 = small.tile([128, NT], fp8, tag="sh8")           # 128*shift, fp8
    for t in range(NT):
        sc = small.tile([128, 1], f32, tag=f"scale{t}")
        nc.vector.tensor_tensor(sc[:], gnw_t[t], ps_bc[:, 4 + t:5 + t],
                                op=ALU.mult)
        sc6 = small.tile([128, 1], f32, tag=f"sc6{t}")
        nc.vector.tensor_scalar_mul(sc6[:], sc[:], 2.0 ** -6)
        nsc = small.tile([128, 1], f32, tag=f"nscale{t}")
        nc.vector.tensor_scalar_mul(nsc[:], sc[:], -1.0)
        sh = small.tile([128, 1], f32, tag=f"shift{t}")
        nc.vector.scalar_tensor_tensor(sh[:], ps_bc[:, t:t + 1], nsc[:],
                                       gnb_t[t], op0=ALU.mult, op1=ALU.add)
        nc.vector.tensor_scalar_mul(sh8[:, t:t + 1], sh[:], 128.0)
        scale_t.append(sc)
        sc6_t.append(sc6)

    # bias chains (N=1 fp8 matmuls, before the in-place weight folds):
    # bq2 = 32(Wq shift + bq);  vc = 2^5 (Wv shift + bv);
    # badd = Wp vc / 2^10  (-> residual constant Wp(Wv shift + bv))
    ps_bq = ps_all[:, 16:20]
    ps_vc = ps_all[:, 20:24]
    for t in range(NT):
        for s in range(NT):
            nc.tensor.matmul(ps_bq[:, t:t + 1],
                             w_q[:, s, t * 128:(t + 1) * 128], sh8[:, s:s + 1],
                             start=False, stop=(s == 3))
            nc.tensor.matmul(ps_vc[:, t:t + 1],
                             w_v[:, s, t * 128:(t + 1) * 128], sh8[:, s:s + 1],
                             start=False, stop=(s == 3))
    if legacy_q:
        bq2 = small.tile([128, 4], f32, tag="bq2")
        for t in range(NT):
            nc.vector.tensor_scalar(bq2[:, t:t + 1], ps_bq[:, t:t + 1],
                                    2.0 ** -7, bq32[t], op0=ALU.mult,
                                    op1=ALU.add)
    else:
        # ub = scale o (M shift) * 32  (chain psum = 4096 * M shift)
        ub = small.tile([128, 4], f32, tag="ub")
        for t in range(NT):
            nc.vector.tensor_scalar(ub[:, t:t + 1], ps_bq[:, t:t + 1],
                                    scale_t[t][:], 2.0 ** -7, op0=ALU.mult,
                                    op1=ALU.mult)
    vc8 = small.tile([128, 4], fp8, tag="vc8")
    for t in range(NT):
        nc.scalar.activation(vc8[:, t:t + 1], ps_vc[:, t:t + 1], AF.Identity,
                             scale=2.0 ** -7, bias=bv32[t])
    ps_t2 = ps_all[:, 24:28]
    for t in range(NT):
        for s in range(NT):
            nc.tensor.matmul(ps_t2[:, t:t + 1],
                             w_p[:, s, t * 128:(t + 1) * 128], vc8[:, s:s + 1],
                             start=False, stop=(s == 3))
    badd = small.tile([128, 4], f32, tag="badd")
    for t in range(NT):
        nc.vector.tensor_scalar(badd[:, t:t + 1], ps_t2[:, t:t + 1],
                                2.0 ** -10, bp_t[t], op0=ALU.mult, op1=ALU.add)

    # fold GN scale into wq / wv (per-partition, in place, Pool)
    for s in range(NT):
        nc.gpsimd.tensor_scalar_mul(w_q[:, s, :], w_q[:, s, :], scale_t[s][:])
        nc.gpsimd.tensor_scalar_mul(w_v[:, s, :], w_v[:, s, :], scale_t[s][:])

    # residual slab + per-channel constant (bp + Wp(Wv shift + bv))
    x_sl = sb.tile([128, NT, QS], f32, tag="x_sl")
    nc.sync.dma_start(x_sl[:], xsl_d.ap().rearrange("(t p) n -> p t n", p=128))
    for t in range(NT):
        nc.gpsimd.tensor_scalar_add(x_sl[:, t, :], x_sl[:, t, :],
                                    badd[:, t:t + 1])

    # ---- phase 1: u (drains alternate ACT/DVE) -------------------
    # Fast path (bq == 0): host sends M = Wk^T Wq in the q slot, so
    # u = scale o (M (scale o x + shift)) comes straight from x with no q
    # projection; the shift term lands in the u-drain bias via the chain.
    # Legacy path (bq != 0): q = Wq h + bq is materialized, then
    # u = Wk^T q (the bq^T k score term is not softmax-invariant).
    u_f8 = sb.tile([128, NT, QS], fp8, tag="u_f8")
    if legacy_q:
        q_f8 = sb.tile([128, NT, QS], fp8, tag="q_f8")
        for t in range(NT):
            psq = ps_mm.tile([128, 1024], f32, tag="mm")
            for qc2 in range(4):
                for a in range(2):
                    nc.tensor.matmul(psq[:, qc2 * 256:(qc2 + 1) * 256],
                                     w_q[:, 2 * a:2 * a + 2, t * 128:(t + 1) * 128],
                                     xpair[a][:, :, qc2 * 256:(qc2 + 1) * 256],
                                     start=(a == 0 and qc2 % 2 == 0),
                                     stop=(a == 1), perf_mode=DR)
            if t % 2 == 0:
                nc.scalar.activation(q_f8[:, t, :], psq[:], AF.Identity,
                                     bias=bq2[:, t:t + 1])
            else:
                nc.vector.tensor_scalar_add(q_f8[:, t, :], psq[:],
                                            bq2[:, t:t + 1])
        for t in range(NT):
            psu = ps_mm.tile([128, 1024], f32, tag="mm")
            for qc2 in range(4):
                for a in range(2):
                    nc.tensor.matmul(psu[:, qc2 * 256:(qc2 + 1) * 256],
                                     w_kT[:, 2 * a:2 * a + 2, t * 128:(t + 1) * 128],
                                     q_f8[:, 2 * a:2 * a + 2, qc2 * 256:(qc2 + 1) * 256],
                                     start=(a == 0 and qc2 % 2 == 0),
                                     stop=(a == 1), perf_mode=DR)
            if t % 2 == 0:
                nc.scalar.mul(u_f8[:, t, :], psu[:], sc6_t[t][:])
            else:
                nc.vector.tensor_scalar_mul(u_f8[:, t, :], psu[:], sc6_t[t][:])
    else:
        for t in range(NT):
            psu = ps_mm.tile([128, 1024], f32, tag="mm")
            for qc2 in range(4):
                for a in range(2):
                    nc.tensor.matmul(psu[:, qc2 * 256:(qc2 + 1) * 256],
                                     w_q[:, 2 * a:2 * a + 2, t * 128:(t + 1) * 128],
                                     xpair[a][:, :, qc2 * 256:(qc2 + 1) * 256],
                                     start=(a == 0 and qc2 % 2 == 0),
                                     stop=(a == 1), perf_mode=DR)
            if t % 2 == 0:
                nc.scalar.activation(u_f8[:, t, :], psu[:], AF.Identity,
                                     scale=scale_t[t][:], bias=ub[:, t:t + 1])
            else:
                nc.vector.tensor_scalar(u_f8[:, t, :], psu[:], scale_t[t][:],
                                        ub[:, t:t + 1], op0=ALU.mult,
                                        op1=ALU.add)

    sexp = SEXP_LEGACY if legacy_q else SEXP_FAST

    # ---- phase 2 ---------------------------------------------------
    # v-projection is interleaved into qch 0 (the vt tiles for quad qd are
    # drained two quads before their o-matmuls need them). Quads are
    # software-pipelined with a skew of 2 (o/sums of quad qd-2 are emitted
    # after scores+exp of quad qd) and each qch's scalar tail (reciprocal,
    # o drain, proj, residual) is deferred until two quads into the NEXT
    # qch, so PE/ACT never sit in the serial tail chain.
    vt_f8 = sb.tile([128, N // 128, C], fp8, tag="vt_f8")

    def emit_v(np_, eng):
        psv = ps_mm.tile([128, 2, C], f32, tag="mm")
        for j in range(2):
            nt = 2 * np_ + j
            for co in range(2):
                for a in range(2):
                    nc.tensor.matmul(psv[:, j, co * 256:(co + 1) * 256],
                                     xpair[a][:, :, nt * 128:(nt + 1) * 128],
                                     w_v[:, 2 * a:2 * a + 2, co * 256:(co + 1) * 256],
                                     start=(co == 0 and a == 0), stop=(a == 1),
                                     perf_mode=DR)
        if eng == "dve":
            nc.vector.tensor_copy(vt_f8[:, 2 * np_:2 * np_ + 2, :], psv[:])
        else:
            nc.scalar.copy(vt_f8[:, 2 * np_:2 * np_ + 2, :], psv[:])

    def flush(st, pp_q, qd):
        ob, sums_ps = st
        for a2 in range(2):
            kt0 = qd * 4 + 2 * a2
            first = (qd == 0 and a2 == 0)
            last = (qd == 7 and a2 == 1)
            for ct in range(4):
                nc.tensor.matmul(ob[ct // 2][:, ct % 2, :],
                                 vt_f8[:, kt0:kt0 + 2, ct * 128:(ct + 1) * 128],
                                 pp_q[:, 2 * a2:2 * a2 + 2, :],
                                 start=(first and ct % 2 == 0), stop=last,
                                 perf_mode=DR)
            nc.tensor.matmul(sums_ps[:, 0:QC], ones4[:],
                             pp_q[:, 2 * a2:2 * a2 + 2, :],
                             start=first, stop=last, perf_mode=DR)

    def tail_a(st):
        ob, sums_ps = st
        r_sb = small.tile([1, QC], f32, name="r_sb", tag="r_sb")
        nc.vector.reciprocal(r_sb[:], sums_ps[0:1, 0:QC])
        r_bf = small.tile([1, QC], bf16, name="r_bf", tag="r_bf")
        nc.vector.tensor_copy(r_bf[:], r_sb[:])
        ps_r = ps_mm.tile([128, 1024], f32, name="ps_r", tag="mm")
        nc.tensor.matmul(ps_r[:, 0:QC], ones_row[:], r_bf[:],
                         start=True, stop=True)
        r_bc2 = small.tile([128, 2, QC], f32, name="r_bc2", tag="r_bc2")
        nc.vector.tensor_copy(r_bc2[:, 0, :], ps_r[:, 0:QC])
        nc.vector.tensor_copy(r_bc2[:, 1, :], ps_r[:, 0:QC])
        return r_bc2

    def tail_b(qch, st, r_bc2):
        ob, sums_ps = st
        q0 = qch * QC
        # o drain fuses softmax normalization: o_f8 = ob * r (= 8 * o_norm)
        o_f8 = scr.tile([128, 4, QC], fp8, name="o_f8", tag="o_f8")
        nc.vector.tensor_tensor(o_f8[:, 0:2, :], ob[0][:], r_bc2[:], op=ALU.mult)
        nc.vector.tensor_tensor(o_f8[:, 2:4, :], ob[1][:], r_bc2[:], op=ALU.mult)
        for half in range(2):
            ppb = ps_o.tile([128, 2, QC], f32, name=f"pp{half}", tag=f"ob{half}")
            for tl in range(2):
                t = 2 * half + tl
                for a in range(2):
                    nc.tensor.matmul(ppb[:, tl, :],
                                     w_p[:, 2 * a:2 * a + 2, t * 128:(t + 1) * 128],
                                     o_f8[:, 2 * a:2 * a + 2, :],
                                     start=(tl == 0 and a == 0), stop=(a == 1),
                                     perf_mode=DR)
            outb = outp.tile([128, 2, QC], f32, name="outb", tag="outb")
            nc.vector.scalar_tensor_tensor(
                outb[:], ppb[:], OSC,
                x_sl[:, 2 * half:2 * half + 2, q0:q0 + QC],
                op0=ALU.mult, op1=ALU.add)
            for tl in range(2):
                nc.sync.dma_start(
                    out.ap()[(2 * half + tl) * 128:(2 * half + tl + 1) * 128,
                             q0:q0 + QC],
                    outb[:, tl, :])

    sts = {}
    rbc = {}
    fq = []

    def pop_flush():
        fqch, fqd, fp = fq.pop(0)
        if fqch not in sts:
            sts[fqch] = ([ps_o.tile([128, 2, QC], f32, name=f"ob{h}",
                                    tag=f"ob{h}") for h in range(2)],
                         ps_sums.tile([32, 512], f32, name="sums", tag="sums"))
        flush(sts[fqch], fp, fqd)

    for qch in range(QS // QC):
        q0 = qch * QC
        for qd in range(8):
            if qch == 0:
                emit_v(2 * qd, "dve")
                emit_v(2 * qd + 1, "act" if qd % 2 == 0 else "dve")
            stq = ps_mm.tile([128, 4, QC], f32, name="stq", tag="mm")
            for j in range(4):
                kt = qd * 4 + j
                for a in range(2):
                    nc.tensor.matmul(stq[:, j, :],
                                     xpair[a][:, :, kt * 128:(kt + 1) * 128],
                                     u_f8[:, 2 * a:2 * a + 2, q0:q0 + QC],
                                     start=(j % 2 == 0 and a == 0), stop=(a == 1),
                                     perf_mode=DR)
            p_q = pq.tile([128, 4, QC], fp8, name="p_q", tag="p")
            nc.scalar.activation(p_q[:], stq[:], AF.Exp, scale=sexp)
            if qch > 0 and qd == 0:
                rbc[qch - 1] = tail_a(sts[qch - 1])
            if qch > 0 and qd == 3:
                tail_b(qch - 1, sts[qch - 1], rbc[qch - 1])
            fq.append((qch, qd, p_q))
            if len(fq) > 3:
                pop_flush()
        if qch == QS // QC - 1:
            while fq:
                pop_flush()
    rbc[3] = tail_a(sts[3])
    tail_b(3, sts[3], rbc[3])


def _make_pools(tc, ctx):
    p = {}
    p["sb"] = ctx.enter_context(tc.tile_pool(name="sb", bufs=1))
    p["scr"] = ctx.enter_context(tc.tile_pool(name="scr", bufs=2))
    p["pq"] = ctx.enter_context(tc.tile_pool(name="pq", bufs=4))
    p["outp"] = ctx.enter_context(tc.tile_pool(name="outp", bufs=2))
    p["small"] = ctx.enter_context(tc.tile_pool(name="small", bufs=1))
    p["ps_mm"] = ctx.enter_context(tc.tile_pool(name="ps_mm", bufs=2, space="PSUM"))
    p["ps_o"] = ctx.enter_context(tc.tile_pool(name="ps_o", bufs=1, space="PSUM"))
    p["ps_sums"] = ctx.enter_context(tc.tile_pool(name="ps_sums", bufs=1, space="PSUM"))
    p["ps_sm"] = ctx.enter_context(tc.tile_pool(name="ps_sm", bufs=1, space="PSUM"))
    return p


def _build(legacy_q=False):
    nc = bacc.Bacc("TRN2", target_bir_lowering=False, debug=False, num_devices=8)
    x8_d = nc.dram_tensor("x8", [C, N], fp8, kind="ExternalInput")
    xsl_d = nc.dram_tensor("xsl", [C, QS], f32, kind="ExternalInput")
    wall_d = nc.dram_tensor("wall", [128, 4 * NT * C], fp8, kind="ExternalInput")
    misc_d = nc.dram_tensor("misc", [128, 672], f32, kind="ExternalInput")
    out = nc.dram_tensor("out", [C, QS], f32, kind="ExternalOutput")
    with tile.TileContext(nc) as tc:
        with contextlib.ExitStack() as ctx:
            p = _make_pools(tc, ctx)
            _emit_body(nc, tc, p, x8_d, xsl_d, wall_d, misc_d, out, legacy_q)
    nc.compile()
    return nc


_NC = {}


def _get_nc(legacy_q=False):
    if legacy_q not in _NC:
        _NC[legacy_q] = _build(legacy_q)
    return _NC[legacy_q]


def kernel(x, gn_w, gn_b, wq, bq, wk, bk, wv, bv, wp, bp):
    x = np.asarray(x, dtype=np.float32)
    B = x.shape[0]
    assert x.shape == (B, C, 64, 64)

    # Fast path (bq == 0): q slot carries M = Wk^T Wq so scores come
    # straight from x. Legacy: q slot = Wq, k slot = Wk untransposed
    # (stationary of u = Wk^T q).
    legacy = bool(np.any(np.asarray(bq, np.float32) != 0.0))
    q_slot = (np.asarray(wk, np.float32).T @ np.asarray(wq, np.float32)
              if not legacy else np.asarray(wq, np.float32))
    wall = np.zeros((128, 4, NT, C), F8)
    for iw, (w, tr) in enumerate(((q_slot, True), (wk, False), (wv, True),
                                 (wp, True))):
        wm = np.asarray(w, np.float32) * SW
        if tr:
            wm = wm.T
        wall[:, iw, :, :] = wm.reshape(NT, 128, C).transpose(1, 0, 2).astype(F8)
    wall = wall.reshape(128, 4 * NT * C)

    misc = np.zeros((128, 672), np.float32)
    for t in range(NT):
        sl = slice(t * 128, (t + 1) * 128)
        misc[:, t] = np.asarray(bq, np.float32)[sl] * SW
        misc[:, 4 + t] = np.asarray(bv, np.float32)[sl] * SW
        misc[:, 8 + t] = np.asarray(bp, np.float32)[sl]
        misc[:, 12 + t] = np.asarray(gn_w, np.float32)[sl]
        misc[:, 16 + t] = np.asarray(gn_b, np.float32)[sl]
    sel8 = np.zeros((128, 8), np.float32)
    for pp_ in range(128):
        sel8[pp_, pp_ // GP] = 1.0
    misc[:, 20:148] = np.concatenate(
        [sel8, np.zeros((128, 120), np.float32)], axis=1)
    misc[0:8, 160:288] = sel8.T

    xf = x.reshape(B, C, N)
    in_maps = []
    for core in range(8):
        b, slab = core // 4, core % 4
        xr = np.roll(xf[b], -QS * slab, axis=1)
        in_maps.append({
            "x8": np.ascontiguousarray(xr).astype(F8),
            "xsl": np.ascontiguousarray(xr[:, 0:QS]),
            "wall": wall, "misc": misc,
        })

    nc = _get_nc(legacy)
    res = bass_utils.run_bass_kernel_spmd(nc, in_maps, core_ids=list(range(8)))

    out = np.empty((B, C, N), np.float32)
    for core in range(8):
        b, slab = core // 4, core % 4
        out[b][:, QS * slab:QS * (slab + 1)] = res.results[core]["out"]
    return out.reshape(B, C, 64, 64)


if __name__ == "__main__":
    rng = np.random.default_rng(0)
    inputs = {
        "x": rng.standard_normal((2, C, 64, 64)).astype(np.float32),
        "gn_w": np.ones(C, np.float32),
        "gn_b": np.zeros(C, np.float32),
    }
    for nm in ("q", "k", "v", "p"):
        inputs[f"w{nm}"] = (rng.standard_normal((C, C)) * 0.02).astype(np.float32)
        inputs[f"b{nm}"] = np.zeros(C, np.float32)
    out = kernel(**inputs)
    print("ran:", out.shape, out.dtype)
